# revision 5
# baseline (speedup 1.0000x reference)
"""Multi-head causal self-attention (B=32, S=512, E=768, H=12, D=64) on 8 TRN2 cores.

Sharding: pure data-parallel over batch (4 batches per core), no collectives.

Per-core layout strategy:
  - x is fed pre-transposed (feature-major) as xT [E, 2048tok].
  - Q^T, K^T are computed feature-major per head-pair (feature tile == head
    pair):  QT_hp = Wq[:, hp].T @ xT   (lhsT=Wq slice, rhs=xT)
  - V is computed token-major with an extra all-ones column per head
    ("V_aug" [tok, H*(D+1)]); the ones column makes the P@V matmul also
    produce the softmax denominators.
  - scores^T[k,q] = K Q^T computed per (head, k-tile of 128 tokens) with the
    causal-trimmed q range [128*i, 512), both heads of a pair packed into the
    128x128 PE array via tile_position row groups.
  - exp() on ScalarE reads score PSUM directly (1/sqrt(D) folded into exp's
    scale), both heads in one call; the causal mask is a post-exp 0/1
    multiply of just the diagonal 128x128 block on VectorE, kept OFF the
    PE->ACT critical path.
  - P@V: out[q, D+1] accumulated over k-tiles i<=j in PSUM; reciprocal of
    column D (the ones-column sum = softmax denominator) normalizes via a
    ScalarE copy with per-partition scale.
  - Y (token-major) is transposed 128x128 via TensorE back to feature-major
    for the output projection, which lands token-major for a contiguous DMA.
  - Emission is software-pipelined (scores of head-pair hp+1 before the PV
    block of hp; next batch's xT DMA prefetched mid-batch) so the in-order
    engine streams always have independent matmuls to hide the cross-engine
    softmax chains.

Dtype strategy (PE cost = moving-dim size x cycles/row; fp32=4, fp32r=1 only
when moving>=256, bf16=1 always, fp8e4+DoubleRow=0.5):
  - V / O projections: fp32r operands (moving dims 384 -> already 1 cyc/row).
  - Q/K projections: fp8e4m3 operands with MatmulPerfMode.DoubleRow, feeding
    two 128-row k-subtiles per PE pass ([P, 2, *] slices of the [P, KT, *]
    layout).  Dominant error source: ~1.2e-2 absmax-rel end to end (gate is
    2e-2).  QK_FP8=0 falls back to fp32r (error ~2e-3).
  - Attention path (Q^T/K^T tiles, exp output P, V tiles, Y, Wo): bf16.
    This makes every PV matmul (free=65) and nq=128 score tile 1 cyc/row.
  - exp() reads f32 score PSUM, emits bf16; softmax normalization is a packed
    per-head-pair reciprocal + broadcast multiply on DVE.
  - hw_loop timing programs wrap a LOOP_UNROLL (default 2) iteration body in
    For_i to amortize the ~32us loop-boundary sync.
Set BASS_MM_F32=1 + QK_FP8=0 for a strict-fp32 fallback.
"""

import os
import sys

import numpy as np

for _p in ("/opt/trn_rl_repo", "/opt/trn_rl_repo/concourse"):
    if _p not in sys.path:
        sys.path.insert(0, _p)

import concourse.bass as bass
import concourse.bacc as bacc
import concourse.mybir as mybir
import concourse.tile as tile

P = 128
E = 768
S = 512
H = 12
D = 64
HP = H // 2          # head pairs
KT = E // P          # 6 feature k-tiles
N_CORES = 8
B_FULL = 32
B_CORE = B_FULL // N_CORES   # 4 batches per core
TOK = B_CORE * S             # 2048 tokens per core
ST = S // P                  # 4 token tiles per sequence
NEG = -1.0e6                 # pre-scale mask bias; exp(0.125 * -1e6) == 0
F32 = mybir.dt.float32

# number of 384-wide chunks for the V / O projections
CH = 2
CHW = E // CH  # 384


def build_program(with_bias: bool, repeat: int = 1, hw_loop: bool = False,
                  r_proj: bool = False, r_scores: bool = False, phases: int = 3,
                  att_bf16: bool = True, qk_fp8: bool | None = None):
    if qk_fp8 is None:
        qk_fp8 = USE_QK_FP8
    PDT = mybir.dt.float32r if r_proj else F32   # proj operands (x, weights)
    BF16 = mybir.dt.bfloat16
    FP8 = mybir.dt.float8e4
    # attention-path operand dtype: qt/kt (scores), pt/md (probs), vs (values),
    # yst/yt (attention out) and wo.  bf16 gets 1 PE cycle/row on ALL matmul
    # shapes (fp32 is 4; fp32r is 4 whenever the moving dim < 256, which hits
    # every PV matmul [free=65] and the nq=128 score tiles).
    ADT = BF16 if att_bf16 else (mybir.dt.float32r if r_scores else F32)
    # fp32r for scores / transpose / oproj: f32(r) matmuls self-load their
    # weights (no separate InstLdweights), cutting ~480 PE instructions per
    # iteration.  Measured +10us on HW (= its exec-cycle cost): the PE is
    # exec-cycle bound, not dispatch bound, so this stays OFF.
    lowinst = os.environ.get("LOW_INST", "0") == "1" and att_bf16
    # fp8 DoubleRow scores: qt/kt stored [P, 2, S] e4m3 with subtile 1
    # pre-zeroed; numerically correct on HW but measured +20us (the doubled
    # moving operand streams at full length), so this stays OFF.
    sc_fp8 = (os.environ.get("SC_FP8", "0") == "1") and qk_fp8 and not lowinst
    # route i=0 score tiles through ps_mm to break the ps_sc serial chain
    sc_split0 = os.environ.get("SC_SPLIT0", "0") == "1" and not sc_fp8
    # per-head [P,S] score tiles, bufs=2 in the same 2 PSUM banks: the two
    # head chains alternate banks, halving the serial scores->exp backbone
    sc_perhead = os.environ.get("SC_PERHEAD", "1") == "1" and not sc_fp8 and not sc_split0
    SCDT = mybir.dt.float32r if lowinst else ADT   # qt/kt (scores operands)
    YSTDT = F32 if lowinst else ADT                # normalize out / transpose in
    YTDT = mybir.dt.float32r if lowinst else ADT   # yt (oproj stationary)
    WODT = (mybir.dt.float32r if lowinst else BF16) if att_bf16 else PDT
    # bf16 x + Wv: halves the per-iteration xt DMA; vproj stays 1 cyc/row.
    xv_bf16 = os.environ.get("XV_BF16", "0") == "1" and att_bf16
    XDT = BF16 if xv_bf16 else PDT
    WVDT = BF16 if xv_bf16 else PDT
    nc = bacc.Bacc(None)
    _eng = {"dve": nc.vector, "act": nc.scalar, "pool": nc.gpsimd, "any": nc.any}
    MASK_ENG = _eng[os.environ.get("MASK_ENG", "dve")].tensor_mul
    MEMSET_ENG = _eng[os.environ.get("MEMSET_ENG", "pool")].memset
    QKCP = _eng[os.environ.get("QKCP_ENG", "any")].tensor_copy
    VCP = _eng[os.environ.get("VCP_ENG", "any")].tensor_copy
    YCP = _eng[os.environ.get("YCP_ENG", "any")].tensor_copy
    OCP = _eng[os.environ.get("OCP_ENG", "any")].tensor_copy
    PV_DIAG_FIRST = os.environ.get("PV_DIAG_FIRST", "0") == "1"

    xt_d = nc.dram_tensor("xt", [E, TOK], XDT, kind="ExternalInput")
    _wnames = ("wv", "wo") if qk_fp8 else ("wq", "wk", "wv", "wo")
    _wdt = {"wq": PDT, "wk": PDT, "wv": WVDT, "wo": WODT}
    w_d = {
        n: nc.dram_tensor(n, [E, E], _wdt[n], kind="ExternalInput")
        for n in _wnames
    }
    consts_d = nc.dram_tensor("consts", [P, 3 * P], F32, kind="ExternalInput")
    if att_bf16:
        # bf16 identity (PE transpose moving operand) + bf16 causal 0/1 mask
        cb_d = nc.dram_tensor("cb", [P, 2 * P], BF16, kind="ExternalInput")
    if qk_fp8:
        xt8_d = nc.dram_tensor("xt8", [E, TOK], FP8, kind="ExternalInput")
        w8_d = {
            n: nc.dram_tensor(n + "8", [E, E], FP8, kind="ExternalInput")
            for n in ("wq", "wk")
        }
    if with_bias:
        bqk_d = nc.dram_tensor("bqk", [P, 2 * KT], F32, kind="ExternalInput")
        bv_d = nc.dram_tensor("bvb", [P, H * (D + 1)], F32, kind="ExternalInput")
        bo_d = nc.dram_tensor("bob", [P, E], F32, kind="ExternalInput")
    y_d = nc.dram_tensor("y", [TOK, E], F32, kind="ExternalOutput")

    with tile.TileContext(nc) as tc:
        with (
            tc.tile_pool(name="wpool", bufs=1) as wpool,
            tc.tile_pool(name="xpool", bufs=2) as xpool,
            tc.tile_pool(name="qkpool", bufs=int(os.environ.get("B_QK", "3"))) as qkpool,
            tc.tile_pool(name="vpool", bufs=int(os.environ.get("B_VS", "2"))) as vpool,
            tc.tile_pool(name="ppool", bufs=int(os.environ.get("B_PT", "8"))) as ppool,
            tc.tile_pool(name="mdpool", bufs=int(os.environ.get("B_MD", "8"))) as mdpool,
            tc.tile_pool(name="ypool", bufs=4) as ypool,
            tc.tile_pool(name="ytpool", bufs=2) as ytpool,
            tc.tile_pool(name="opool", bufs=2) as opool,
            tc.tile_pool(name="rpool", bufs=4) as rpool,
            tc.tile_pool(name="ps_mm", bufs=int(os.environ.get("B_MM", "3")), space="PSUM") as ps_mm,
            tc.tile_pool(name="ps_sc", bufs=int(os.environ.get("B_SC", "2" if (os.environ.get("SC_PERHEAD", "1") == "1") else "1")), space="PSUM") as ps_sc,
            tc.tile_pool(name="ps_pv", bufs=int(os.environ.get("B_PV", "2")), space="PSUM") as ps_pv,
            tc.tile_pool(name="ps_yt", bufs=int(os.environ.get("B_YT", "1")), space="PSUM") as ps_yt,
        ):
            # ---- persistent constants ----
            w_sb = {}
            for n in _wnames:
                t = wpool.tile([P, KT, E], _wdt[n], tag=n)
                nc.sync.dma_start(t[:], w_d[n][:].rearrange("(ko ki) m -> ki ko m", ki=P))
                w_sb[n] = t
            cons = wpool.tile([P, 3 * P], F32, tag="consts")  # masks stay f32
            nc.sync.dma_start(cons[:], consts_d[:])
            ident = cons[:, 0:P]
            mask01 = cons[:, 2 * P : 3 * P]
            if att_bf16:
                cb = wpool.tile([P, 2 * P], mybir.dt.bfloat16, tag="cb")
                nc.sync.dma_start(cb[:], cb_d[:])
                if not lowinst:
                    ident = cb[:, 0:P]
                mask01 = cb[:, P : 2 * P]
            w8_sb = {}
            if qk_fp8:
                for n in ("wq", "wk"):
                    t = wpool.tile([P, KT, E], FP8, tag=n + "8")
                    nc.sync.dma_start(
                        t[:], w8_d[n][:].rearrange("(ko ki) m -> ki ko m", ki=P)
                    )
                    w8_sb[n] = t
            if with_bias:
                bqk = wpool.tile([P, 2 * KT], F32, tag="bqk")
                nc.sync.dma_start(bqk[:], bqk_d[:])
                bvb = wpool.tile([P, H * (D + 1)], F32, tag="bvb")
                nc.sync.dma_start(bvb[:], bv_d[:])
                bob = wpool.tile([P, E], F32, tag="bob")
                nc.sync.dma_start(bob[:], bo_d[:])

            if sc_fp8:
                # pre-zero subtile 1 of every qk pool buffer once; the live
                # copies only ever write subtile 0, so these zeros persist
                for _ in range(int(os.environ.get("B_QK", "3"))):
                    for tag in ("qt", "kt"):
                        tz = qkpool.tile([P, 2, S], FP8, tag=tag, name="tz")
                        MEMSET_ENG(tz[:, 1, :], 0.0)

            xt_r = xt_d[:].rearrange("(ko ki) t -> ki ko t", ki=P)
            if qk_fp8:
                xt8_r = xt8_d[:].rearrange("(ko ki) t -> ki ko t", ki=P)

            xts_t = {}

            def load(pos, b):
                tok0 = (b % B_CORE) * S
                xts = xpool.tile([P, KT, S], XDT, tag="xts")
                nc.sync.dma_start(xts[:], xt_r[:, :, tok0 : tok0 + S])
                xts8 = None
                if qk_fp8:
                    xts8 = xpool.tile([P, KT, S], FP8, tag="xts8")
                    nc.sync.dma_start(xts8[:], xt8_r[:, :, tok0 : tok0 + S])
                xts_t[pos] = (xts, xts8)

            def vproj(b, xts):
                # ---- V projection (token-major, augmented with ones cols) ----
                # k outer / ch inner: the two ch matmuls share the same
                # stationary (xts k-slice), so legalization skips every other
                # InstLdweights (48 -> 24 weight loads per batch).
                vs = []
                for tt in range(ST):
                    v_t = vpool.tile([P, H, D + 1], ADT, tag=f"vs{tt}")
                    MEMSET_ENG(v_t[:, :, D : D + 1], 1.0)
                    pss = [ps_mm.tile([P, S], F32, tag="mm", name=f"psv{c}")
                           for c in range(CH)]
                    for k in range(KT):
                        for ch in range(CH):
                            nc.tensor.matmul(
                                pss[ch][:, :CHW],
                                xts[:, k, tt * P : (tt + 1) * P],
                                w_sb["wv"][:, k, ch * CHW : (ch + 1) * CHW],
                                start=(k == 0),
                                stop=(k == KT - 1),
                            )
                    for ch in range(CH):
                        psc = pss[ch][:, :CHW]
                        hpc = CHW // D  # heads per chunk (6)
                        dst = v_t[:, ch * hpc : (ch + 1) * hpc, 0:D]
                        VCP(out=dst, in_=psc.rearrange("p (h d) -> p h d", d=D))
                    if with_bias:
                        nc.vector.tensor_add(
                            out=v_t[:],
                            in0=v_t[:],
                            in1=bvb[:].rearrange("p (h d) -> p h d", d=D + 1),
                        )
                    vs.append(v_t)
                return vs

            def qk_scores(b, xts, xts8, hp):
                # Q^T / K^T for this head pair (feature tile hp)
                qk = {}
                for name, tag in (("wq", "qt"), ("wk", "kt")):
                    if sc_fp8:
                        dst = qkpool.tile([P, 2, S], FP8, tag=tag)
                    else:
                        dst = qkpool.tile([P, S], SCDT, tag=tag)
                    ps = ps_mm.tile([P, S], F32, tag="mm")
                    if qk_fp8:
                        # fp8 DoubleRow: two 128-row k-subtiles per pass
                        for k in range(0, KT, 2):
                            nc.tensor.matmul(
                                ps[:],
                                w8_sb[name][:, k : k + 2, hp * P : (hp + 1) * P],
                                xts8[:, k : k + 2, :],
                                start=(k == 0),
                                stop=(k == KT - 2),
                                perf_mode=mybir.MatmulPerfMode.DoubleRow,
                            )
                    else:
                        for k in range(KT):
                            nc.tensor.matmul(
                                ps[:],
                                w_sb[name][:, k, hp * P : (hp + 1) * P],
                                xts[:, k, :],
                                start=(k == 0),
                                stop=(k == KT - 1),
                            )
                    if with_bias:
                        col = (0 if name == "wq" else KT) + hp
                        nc.vector.tensor_scalar_add(
                            dst[:, 0, :] if sc_fp8 else dst[:], ps[:],
                            bqk[:, col : col + 1],
                        )
                    else:
                        QKCP(out=dst[:, 0, :] if sc_fp8 else dst[:], in_=ps[:])
                    qk[tag] = dst
                qt, kt = qk["qt"], qk["kt"]

                # scores^T + exp, causal-trimmed per k-tile.  With
                # SC_SPLIT0 the i=0 (nq=512) tiles go through ps_mm per head,
                # so the serial scores->exp chain through the single ps_sc
                # bank loses its heaviest link and the two pools alternate.
                pts = []  # pts[i] = exp(scores^T) [P, 2, Nq] (heads of pair)
                for i in range(ST):
                    nq = S - i * P
                    qoff = i * P
                    if sc_split0 and i == 0:
                        pt = ppool.tile([P, 2, S], ADT, tag="pt")
                        for hh in range(2):
                            ro = hh * D
                            psh = ps_mm.tile([P, S], F32, tag="mm")
                            nc.tensor.matmul(
                                psh[:, 0:nq],
                                kt[ro : ro + D, i * P : (i + 1) * P],
                                qt[ro : ro + D, qoff:S],
                                start=True,
                                stop=True,
                                tile_position=(ro, 0),
                            )
                            nc.scalar.activation(
                                pt[:, hh, 0:nq],
                                psh[:, 0:nq],
                                mybir.ActivationFunctionType.Exp,
                                scale=0.125,
                            )
                    elif sc_perhead:
                        pt = ppool.tile([P, 2, S], ADT, tag="pt")
                        for hh in range(2):
                            ro = hh * D
                            psh = ps_sc.tile([P, S], F32, tag="sc")
                            nc.tensor.matmul(
                                psh[:, 0:nq],
                                kt[ro : ro + D, i * P : (i + 1) * P],
                                qt[ro : ro + D, qoff:S],
                                start=True,
                                stop=True,
                                tile_position=(ro, 0),
                            )
                            nc.scalar.activation(
                                pt[:, hh, 0:nq],
                                psh[:, 0:nq],
                                mybir.ActivationFunctionType.Exp,
                                scale=0.125,
                            )
                    else:
                        ps = ps_sc.tile(
                            [P, 2, 3 * P] if sc_split0 else [P, 2, S],
                            F32, tag="sc",
                        )
                        for hh in range(2):
                            ro = hh * D
                            if sc_fp8:
                                nc.tensor.matmul(
                                    ps[:, hh, 0:nq],
                                    kt[ro : ro + D, :, i * P : (i + 1) * P],
                                    qt[ro : ro + D, :, qoff:S],
                                    start=True,
                                    stop=True,
                                    tile_position=(ro, 0),
                                    perf_mode=mybir.MatmulPerfMode.DoubleRow,
                                )
                            else:
                                nc.tensor.matmul(
                                    ps[:, hh, 0:nq],
                                    kt[ro : ro + D, i * P : (i + 1) * P],
                                    qt[ro : ro + D, qoff:S],
                                    start=True,
                                    stop=True,
                                    tile_position=(ro, 0),
                                )
                        pt = ppool.tile([P, 2, S], ADT, tag="pt")
                        nc.scalar.activation(
                            pt[:, :, 0:nq],
                            ps[:, :, 0:nq],
                            mybir.ActivationFunctionType.Exp,
                            scale=0.125,
                        )
                    # causal mask: zero the upper triangle of the diagonal
                    # block, off the PE->ACT critical path (Pool engine,
                    # post-exp; all-SBUF operands so GpSimd can run it)
                    md = mdpool.tile([P, 2, P], ADT, tag="md")
                    MASK_ENG(
                        out=md[:], in0=pt[:, :, 0:P],
                        in1=mask01[:, None, :].to_broadcast((P, 2, P)),
                    )
                    pts.append((pt, md))
                return pts

            TPOST = os.environ.get("TPOST", "0") == "1"

            def pv_j(hp, pts, vs, j):
                yst = ypool.tile([P, 2, D], YSTDT, tag="yst")
                pv = ps_pv.tile([P, 2, D + 1], F32, tag="pv")
                for hh in range(2):
                    h = 2 * hp + hh
                    order = ([j] + list(range(j))) if PV_DIAG_FIRST else range(j + 1)
                    for ii, i in enumerate(order):
                        pt, md = pts[i]
                        lhsT = (
                            md[:, hh, :]
                            if i == j
                            else pt[:, hh, (j - i) * P : (j - i + 1) * P]
                        )
                        nc.tensor.matmul(
                            pv[:, hh, :],
                            lhsT,
                            vs[i][:, h, :],
                            start=(ii == 0),
                            stop=(ii == j),
                        )
                # one packed reciprocal + one broadcast multiply per
                # (head-pair, q-tile) on DVE, replacing 4 ACT/DVE ops
                r = rpool.tile([P, 2], F32, tag="r")
                nc.vector.reciprocal(r[:], pv[:, :, D])
                nc.vector.tensor_mul(
                    out=yst[:],
                    in0=pv[:, :, 0:D],
                    in1=r[:, :, None].to_broadcast((P, 2, D)),
                )
                return yst

            def yst_out(hp, yt, j, yst):
                yt_ps = ps_yt.tile([P, P], YSTDT, tag="ytp")
                nc.tensor.transpose(yt_ps[:], yst[:], ident)
                YCP(out=yt[:, hp, j * P : (j + 1) * P], in_=yt_ps[:])

            def pv_block(hp, pts, vs, yt):
                # P @ V_aug accumulated over k-tiles, then normalize,
                # then transpose Y back to feature-major.  With TPOST the
                # transposes of a j-pair are deferred until after both PV
                # chains so they do not head-of-line-block the PE queue
                # while the DVE normalize completes.
                if TPOST:
                    for jp in range(0, ST, 2):
                        ysts = [(j, pv_j(hp, pts, vs, j)) for j in (jp, jp + 1)]
                        for j, yst in ysts:
                            yst_out(hp, yt, j, yst)
                else:
                    for j in range(ST):
                        yst = pv_j(hp, pts, vs, j)
                        yst_out(hp, yt, j, yst)

            O_DMA = os.environ.get("O_DMA", "0") == "1" and not with_bias

            def oproj_tt(b, yt, tt):
                tok0 = (b % B_CORE) * S
                # k outer / ch inner: both ch matmuls share the stationary
                # (yt k-slice) so half the InstLdweights are elided.
                pss = [ps_mm.tile([P, S], F32, tag="mm", name=f"pso{c}")
                       for c in range(CH)]
                for k in range(KT):
                    for ch in range(CH):
                        nc.tensor.matmul(
                            pss[ch][:, :CHW],
                            yt[:, k, tt * P : (tt + 1) * P],
                            w_sb["wo"][:, k, ch * CHW : (ch + 1) * CHW],
                            start=(k == 0),
                            stop=(k == KT - 1),
                        )
                if O_DMA:
                    # DMA y straight out of PSUM, skipping the SBUF bounce
                    for ch in range(CH):
                        nc.sync.dma_start(
                            y_d[
                                tok0 + tt * P : tok0 + (tt + 1) * P,
                                ch * CHW : (ch + 1) * CHW,
                            ],
                            pss[ch][:, :CHW],
                        )
                else:
                    o_sb = opool.tile([P, E], F32, tag="osb")
                    for ch in range(CH):
                        OCP(out=o_sb[:, ch * CHW : (ch + 1) * CHW], in_=pss[ch][:, :CHW])
                    if with_bias:
                        nc.vector.tensor_add(out=o_sb[:], in0=o_sb[:], in1=bob[:])
                    nc.sync.dma_start(
                        y_d[tok0 + tt * P : tok0 + (tt + 1) * P, :], o_sb[:]
                    )

            def run_batches(batches):
                # Software-pipelined emission: scores of head-pair hp+1 are
                # emitted before the PV block of hp, so the tensor engine's
                # in-order stream always has matmuls to run while the
                # mask(DVE) -> exp(ACT) -> normalize(DVE) chains drain.
                load(0, batches[0])
                pending_o = None  # (b, yt) of the previous batch
                for idx, b in enumerate(batches):
                    xts, xts8 = xts_t.pop(idx)
                    vs = vproj(b, xts)
                    yt = ytpool.tile([P, KT, S], YTDT, tag="yt")
                    pts_next = qk_scores(b, xts, xts8, 0)
                    for hp in range(HP):
                        pts_cur = pts_next
                        # previous batch's output projection, one token tile
                        # at a time, spread through the PV chain gaps
                        if pending_o is not None and hp < ST:
                            oproj_tt(*pending_o, hp)
                        if hp == 2 and idx + 1 < len(batches):
                            load(idx + 1, batches[idx + 1])
                        if hp + 1 < HP:
                            pts_next = qk_scores(b, xts, xts8, hp + 1)
                        pv_block(hp, pts_cur, vs, yt)
                    pending_o = (b, yt)
                for tt in range(ST):
                    oproj_tt(*pending_o, tt)

            # hw_loop body covers `unroll` logical iterations to amortize the
            # For_i boundary sync; repeat must be a multiple of unroll.
            unroll = int(os.environ.get("LOOP_UNROLL", "4"))
            if hw_loop and repeat > 1:
                if repeat % unroll != 0:
                    unroll = 1
                body = [b % B_CORE for b in range(B_CORE * unroll)]
                with tc.For_i(0, repeat // unroll, 1):
                    run_batches(body)
            else:
                run_batches([b % B_CORE for b in range(B_CORE * repeat)])

    nc.compile()
    return nc


def _host_consts():
    ident = np.eye(P, dtype=np.float32)
    k_idx = np.arange(P, dtype=np.int64)[:, None]
    q_idx = np.arange(P, dtype=np.int64)[None, :]
    maskb = np.where(k_idx <= q_idx, 0.0, NEG).astype(np.float32)
    mask01 = (k_idx <= q_idx).astype(np.float32)
    return np.concatenate([ident, maskb, mask01], axis=1)  # [P, 3P]


def _host_consts_bf16():
    import ml_dtypes

    ident = np.eye(P, dtype=np.float32)
    k_idx = np.arange(P, dtype=np.int64)[:, None]
    q_idx = np.arange(P, dtype=np.int64)[None, :]
    mask01 = (k_idx <= q_idx).astype(np.float32)
    return np.concatenate([ident, mask01], axis=1).astype(ml_dtypes.bfloat16)


_PROG_CACHE = {}


# fp32r (relaxed single-pass fp32 matmul, ~2e-4 rel err, 4x PE throughput) is
# used by default; set BASS_MM_F32=1 for strict fp32 matmuls (~2x slower).
USE_F32R = os.environ.get("BASS_MM_F32", "0") != "1"
# fp8e4m3 DoubleRow Q/K projections (2 k-subtiles per PE pass).
USE_QK_FP8 = os.environ.get("QK_FP8", "1") == "1"


def _get_program(with_bias: bool):
    if with_bias not in _PROG_CACHE:
        _PROG_CACHE[with_bias] = build_program(
            with_bias, r_proj=USE_F32R, r_scores=USE_F32R
        )
    return _PROG_CACHE[with_bias]


def make_in_maps(x, Wq, bq, Wk, bk, Wv, bv, Wo, bo, with_bias, att_bf16=True):
    import ml_dtypes

    consts = _host_consts()
    lowinst = os.environ.get("LOW_INST", "0") == "1" and att_bf16
    wo_dt = (np.float32 if lowinst else ml_dtypes.bfloat16) if att_bf16 else np.float32
    xv_bf16 = os.environ.get("XV_BF16", "0") == "1" and att_bf16
    x_dt = ml_dtypes.bfloat16 if xv_bf16 else np.float32
    wv_dt = ml_dtypes.bfloat16 if xv_bf16 else np.float32
    maps = []
    for c in range(N_CORES):
        xc = np.ascontiguousarray(
            x[c * B_CORE : (c + 1) * B_CORE]  # [B_CORE, S, E]
            .reshape(TOK, E)
            .T  # [E, TOK]
        ).astype(np.float32)
        m = {
            "xt": np.ascontiguousarray(xc.astype(x_dt)),
            "wv": np.ascontiguousarray(np.asarray(Wv, np.float32).astype(wv_dt)),
            "wo": np.ascontiguousarray(np.asarray(Wo).astype(wo_dt)),
            "consts": consts,
        }
        if not USE_QK_FP8:
            m["wq"] = np.ascontiguousarray(Wq, dtype=np.float32)
            m["wk"] = np.ascontiguousarray(Wk, dtype=np.float32)
        if att_bf16:
            m["cb"] = _host_consts_bf16()
        if USE_QK_FP8:
            f8 = ml_dtypes.float8_e4m3
            m["xt8"] = np.ascontiguousarray(xc.astype(f8))
            m["wq8"] = np.ascontiguousarray(np.asarray(Wq, np.float32).astype(f8))
            m["wk8"] = np.ascontiguousarray(np.asarray(Wk, np.float32).astype(f8))
        if with_bias:
            bqk = np.concatenate(
                [np.asarray(bq).reshape(KT, P).T, np.asarray(bk).reshape(KT, P).T],
                axis=1,
            ).astype(np.float32)
            bvb = np.zeros((P, H, D + 1), np.float32)
            bvb[:, :, :D] = np.broadcast_to(np.asarray(bv).reshape(H, D), (P, H, D))
            m["bqk"] = np.ascontiguousarray(bqk)
            m["bvb"] = np.ascontiguousarray(bvb.reshape(P, H * (D + 1)))
            m["bob"] = np.ascontiguousarray(
                np.broadcast_to(np.asarray(bo, dtype=np.float32), (P, E))
            )
        maps.append(m)
    return maps


def kernel(x, Wq, bq, Wk, bk, Wv, bv, Wo, bo):
    from concourse.bass_utils import run_bass_kernel_spmd

    x = np.asarray(x, dtype=np.float32)
    with_bias = any(
        float(np.abs(np.asarray(b)).max()) != 0.0 for b in (bq, bk, bv, bo)
    )
    nc = _get_program(with_bias)
    in_maps = make_in_maps(x, Wq, bq, Wk, bk, Wv, bv, Wo, bo, with_bias)
    res = run_bass_kernel_spmd(nc, in_maps, core_ids=list(range(N_CORES)))
    out = np.empty((B_FULL, S, E), dtype=np.float32)
    for c in range(N_CORES):
        out[c * B_CORE : (c + 1) * B_CORE] = res.results[c]["y"].reshape(B_CORE, S, E)
    return out



# revision 9
# speedup vs baseline: 1.1138x; 1.1138x over previous
"""Multi-head causal self-attention (B=32, S=512, E=768, H=12, D=64) on 8 TRN2 cores.

Sharding: pure data-parallel over batch (4 batches per core), no collectives.

Per-core layout strategy:
  - x is fed pre-transposed (feature-major) as xT [E, 2048tok].
  - Q^T, K^T are computed feature-major per head-pair (feature tile == head
    pair):  QT_hp = Wq[:, hp].T @ xT   (lhsT=Wq slice, rhs=xT)
  - V is computed token-major with an extra all-ones column per head
    ("V_aug" [tok, H*(D+1)]); the ones column makes the P@V matmul also
    produce the softmax denominators.
  - scores^T[k,q] = K Q^T computed per (head, k-tile of 128 tokens) with the
    causal-trimmed q range [128*i, 512), both heads of a pair packed into the
    128x128 PE array via tile_position row groups.
  - exp() on ScalarE reads score PSUM directly (1/sqrt(D) folded into exp's
    scale), both heads in one call; the causal mask is a post-exp 0/1
    multiply of just the diagonal 128x128 block on VectorE, kept OFF the
    PE->ACT critical path.
  - P@V: out[q, D+1] accumulated over k-tiles i<=j in PSUM; reciprocal of
    column D (the ones-column sum = softmax denominator) normalizes via a
    ScalarE copy with per-partition scale.
  - Y (token-major) is transposed 128x128 via TensorE back to feature-major
    for the output projection, which lands token-major for a contiguous DMA.
  - Emission is software-pipelined (scores of head-pair hp+1 before the PV
    block of hp; next batch's xT DMA prefetched mid-batch) so the in-order
    engine streams always have independent matmuls to hide the cross-engine
    softmax chains.

Dtype strategy (PE cost = moving-dim size x cycles/row; fp32=4, fp32r=1 only
when moving>=256, bf16=1 always, fp8e4+DoubleRow=0.5):
  - V / O projections: fp32r operands (moving dims 384 -> already 1 cyc/row).
  - Q/K projections: fp8e4m3 operands with MatmulPerfMode.DoubleRow, feeding
    two 128-row k-subtiles per PE pass ([P, 2, *] slices of the [P, KT, *]
    layout).  Dominant error source: ~1.2e-2 absmax-rel end to end (gate is
    2e-2).  QK_FP8=0 falls back to fp32r (error ~2e-3).
  - Attention path (Q^T/K^T tiles, exp output P, V tiles, Y, Wo): bf16.
    This makes every PV matmul (free=65) and nq=128 score tile 1 cyc/row.
  - exp() reads f32 score PSUM, emits bf16; softmax normalization is a packed
    per-head-pair reciprocal + broadcast multiply on DVE.
  - hw_loop timing programs wrap a LOOP_UNROLL (default 2) iteration body in
    For_i to amortize the ~32us loop-boundary sync.  LOOP_UNROLL=4 measured
    +8us/iter on HW (bigger body hurts more than the halved barrier helps;
    likely instruction-fetch locality), so 2 stays the default.
  - V / O projections emit k-outer / ch-inner so consecutive matmuls
    alternate between the two ch PSUM banks (same stationary back to back):
    measured -6.6us/iter on HW vs the ch-outer ordering.
  - SC_PERHEAD=1 (per-head score tiles, 2 PSUM banks) looks -2us in
    TimelineSim but measured +20us/iter on HW: keep OFF.
  - XV_BF16=1 (default ON): x and Wv in bf16 halve the per-iteration xt
    DMA (6.3 -> 3.1 MB).  Measured -7.1us/iter on HW: the 8 cores share
    HBM bandwidth, so DMA volume matters more than single-core sim says.
  - fp8 V or O projections are numerically dead: host-sim absmax-rel 4e-2
    vs the 2e-2 gate (vs 1.2e-2 for the current QK-fp8-only config).
Set BASS_MM_F32=1 + QK_FP8=0 for a strict-fp32 fallback.
"""

import os
import sys

import numpy as np

for _p in ("/opt/trn_rl_repo", "/opt/trn_rl_repo/concourse"):
    if _p not in sys.path:
        sys.path.insert(0, _p)

import concourse.bass as bass
import concourse.bacc as bacc
import concourse.mybir as mybir
import concourse.tile as tile

P = 128
E = 768
S = 512
H = 12
D = 64
HP = H // 2          # head pairs
KT = E // P          # 6 feature k-tiles
N_CORES = 8
B_FULL = 32
B_CORE = B_FULL // N_CORES   # 4 batches per core
TOK = B_CORE * S             # 2048 tokens per core
ST = S // P                  # 4 token tiles per sequence
NEG = -1.0e6                 # pre-scale mask bias; exp(0.125 * -1e6) == 0
F32 = mybir.dt.float32

# number of 384-wide chunks for the V / O projections
CH = 2
CHW = E // CH  # 384


def build_program(with_bias: bool, repeat: int = 1, hw_loop: bool = False,
                  r_proj: bool = False, r_scores: bool = False, phases: int = 3,
                  att_bf16: bool = True, qk_fp8: bool | None = None):
    if qk_fp8 is None:
        qk_fp8 = USE_QK_FP8
    PDT = mybir.dt.float32r if r_proj else F32   # proj operands (x, weights)
    BF16 = mybir.dt.bfloat16
    FP8 = mybir.dt.float8e4
    # attention-path operand dtype: qt/kt (scores), pt/md (probs), vs (values),
    # yst/yt (attention out) and wo.  bf16 gets 1 PE cycle/row on ALL matmul
    # shapes (fp32 is 4; fp32r is 4 whenever the moving dim < 256, which hits
    # every PV matmul [free=65] and the nq=128 score tiles).
    ADT = BF16 if att_bf16 else (mybir.dt.float32r if r_scores else F32)
    # fp32r for scores / transpose / oproj: f32(r) matmuls self-load their
    # weights (no separate InstLdweights), cutting ~480 PE instructions per
    # iteration.  Measured +10us on HW (= its exec-cycle cost): the PE is
    # exec-cycle bound, not dispatch bound, so this stays OFF.
    lowinst = os.environ.get("LOW_INST", "0") == "1" and att_bf16
    # fp8 DoubleRow scores: qt/kt stored [P, 2, S] e4m3 with subtile 1
    # pre-zeroed; numerically correct on HW but measured +20us (the doubled
    # moving operand streams at full length), so this stays OFF.
    sc_fp8 = (os.environ.get("SC_FP8", "0") == "1") and qk_fp8 and not lowinst
    # route i=0 score tiles through ps_mm to break the ps_sc serial chain
    sc_split0 = os.environ.get("SC_SPLIT0", "0") == "1" and not sc_fp8
    # per-head [P,S] score tiles, bufs=2 in the same 2 PSUM banks: the two
    # head chains alternate banks, halving the serial scores->exp backbone
    sc_perhead = os.environ.get("SC_PERHEAD", "0") == "1" and not sc_fp8 and not sc_split0
    SCDT = mybir.dt.float32r if lowinst else ADT   # qt/kt (scores operands)
    YSTDT = F32 if lowinst else ADT                # normalize out / transpose in
    YTDT = mybir.dt.float32r if lowinst else ADT   # yt (oproj stationary)
    WODT = (mybir.dt.float32r if lowinst else BF16) if att_bf16 else PDT
    # bf16 x + Wv: halves the per-iteration xt DMA; vproj stays 1 cyc/row.
    xv_bf16 = os.environ.get("XV_BF16", "1") == "1" and att_bf16
    XDT = BF16 if xv_bf16 else PDT
    WVDT = BF16 if xv_bf16 else PDT
    nc = bacc.Bacc(None)
    _eng = {"dve": nc.vector, "act": nc.scalar, "pool": nc.gpsimd, "any": nc.any}
    MASK_ENG = _eng[os.environ.get("MASK_ENG", "dve")].tensor_mul
    MEMSET_ENG = _eng[os.environ.get("MEMSET_ENG", "pool")].memset
    QKCP = _eng[os.environ.get("QKCP_ENG", "any")].tensor_copy
    VCP = _eng[os.environ.get("VCP_ENG", "any")].tensor_copy
    YCP = _eng[os.environ.get("YCP_ENG", "any")].tensor_copy
    OCP = _eng[os.environ.get("OCP_ENG", "any")].tensor_copy
    PV_DIAG_FIRST = os.environ.get("PV_DIAG_FIRST", "0") == "1"

    xt_d = nc.dram_tensor("xt", [E, TOK], XDT, kind="ExternalInput")
    _wnames = ("wv", "wo") if qk_fp8 else ("wq", "wk", "wv", "wo")
    _wdt = {"wq": PDT, "wk": PDT, "wv": WVDT, "wo": WODT}
    w_d = {
        n: nc.dram_tensor(n, [E, E], _wdt[n], kind="ExternalInput")
        for n in _wnames
    }
    consts_d = nc.dram_tensor("consts", [P, 3 * P], F32, kind="ExternalInput")
    if att_bf16:
        # bf16 identity (PE transpose moving operand) + bf16 causal 0/1 mask
        cb_d = nc.dram_tensor("cb", [P, 2 * P], BF16, kind="ExternalInput")
    if qk_fp8:
        xt8_d = nc.dram_tensor("xt8", [E, TOK], FP8, kind="ExternalInput")
        w8_d = {
            n: nc.dram_tensor(n + "8", [E, E], FP8, kind="ExternalInput")
            for n in ("wq", "wk")
        }
    if with_bias:
        bqk_d = nc.dram_tensor("bqk", [P, 2 * KT], F32, kind="ExternalInput")
        bv_d = nc.dram_tensor("bvb", [P, H * (D + 1)], F32, kind="ExternalInput")
        bo_d = nc.dram_tensor("bob", [P, E], F32, kind="ExternalInput")
    y_d = nc.dram_tensor("y", [TOK, E], F32, kind="ExternalOutput")

    with tile.TileContext(nc) as tc:
        with (
            tc.tile_pool(name="wpool", bufs=1) as wpool,
            tc.tile_pool(name="xpool", bufs=2) as xpool,
            tc.tile_pool(name="qkpool", bufs=int(os.environ.get("B_QK", "3"))) as qkpool,
            tc.tile_pool(name="vpool", bufs=int(os.environ.get("B_VS", "2"))) as vpool,
            tc.tile_pool(name="ppool", bufs=int(os.environ.get("B_PT", "8"))) as ppool,
            tc.tile_pool(name="mdpool", bufs=int(os.environ.get("B_MD", "8"))) as mdpool,
            tc.tile_pool(name="ypool", bufs=4) as ypool,
            tc.tile_pool(name="ytpool", bufs=2) as ytpool,
            tc.tile_pool(name="opool", bufs=2) as opool,
            tc.tile_pool(name="rpool", bufs=4) as rpool,
            tc.tile_pool(name="ps_mm", bufs=int(os.environ.get("B_MM", "3")), space="PSUM") as ps_mm,
            tc.tile_pool(name="ps_sc", bufs=int(os.environ.get("B_SC", "2" if (os.environ.get("SC_PERHEAD", "0") == "1") else "1")), space="PSUM") as ps_sc,
            tc.tile_pool(name="ps_pv", bufs=int(os.environ.get("B_PV", "2")), space="PSUM") as ps_pv,
            tc.tile_pool(name="ps_yt", bufs=int(os.environ.get("B_YT", "1")), space="PSUM") as ps_yt,
        ):
            # ---- persistent constants ----
            w_sb = {}
            for n in _wnames:
                t = wpool.tile([P, KT, E], _wdt[n], tag=n)
                nc.sync.dma_start(t[:], w_d[n][:].rearrange("(ko ki) m -> ki ko m", ki=P))
                w_sb[n] = t
            cons = wpool.tile([P, 3 * P], F32, tag="consts")  # masks stay f32
            nc.sync.dma_start(cons[:], consts_d[:])
            ident = cons[:, 0:P]
            mask01 = cons[:, 2 * P : 3 * P]
            if att_bf16:
                cb = wpool.tile([P, 2 * P], mybir.dt.bfloat16, tag="cb")
                nc.sync.dma_start(cb[:], cb_d[:])
                if not lowinst:
                    ident = cb[:, 0:P]
                mask01 = cb[:, P : 2 * P]
            w8_sb = {}
            if qk_fp8:
                for n in ("wq", "wk"):
                    t = wpool.tile([P, KT, E], FP8, tag=n + "8")
                    nc.sync.dma_start(
                        t[:], w8_d[n][:].rearrange("(ko ki) m -> ki ko m", ki=P)
                    )
                    w8_sb[n] = t
            if with_bias:
                bqk = wpool.tile([P, 2 * KT], F32, tag="bqk")
                nc.sync.dma_start(bqk[:], bqk_d[:])
                bvb = wpool.tile([P, H * (D + 1)], F32, tag="bvb")
                nc.sync.dma_start(bvb[:], bv_d[:])
                bob = wpool.tile([P, E], F32, tag="bob")
                nc.sync.dma_start(bob[:], bo_d[:])

            if sc_fp8:
                # pre-zero subtile 1 of every qk pool buffer once; the live
                # copies only ever write subtile 0, so these zeros persist
                for _ in range(int(os.environ.get("B_QK", "3"))):
                    for tag in ("qt", "kt"):
                        tz = qkpool.tile([P, 2, S], FP8, tag=tag, name="tz")
                        MEMSET_ENG(tz[:, 1, :], 0.0)

            xt_r = xt_d[:].rearrange("(ko ki) t -> ki ko t", ki=P)
            if qk_fp8:
                xt8_r = xt8_d[:].rearrange("(ko ki) t -> ki ko t", ki=P)

            xts_t = {}

            def load(pos, b):
                tok0 = (b % B_CORE) * S
                xts = xpool.tile([P, KT, S], XDT, tag="xts")
                nc.sync.dma_start(xts[:], xt_r[:, :, tok0 : tok0 + S])
                xts8 = None
                if qk_fp8:
                    xts8 = xpool.tile([P, KT, S], FP8, tag="xts8")
                    nc.sync.dma_start(xts8[:], xt8_r[:, :, tok0 : tok0 + S])
                xts_t[pos] = (xts, xts8)

            def vproj(b, xts):
                # ---- V projection (token-major, augmented with ones cols) ----
                # k outer / ch inner: the two ch matmuls share the same
                # stationary (xts k-slice), so legalization skips every other
                # InstLdweights (48 -> 24 weight loads per batch).
                vs = []
                for tt in range(ST):
                    v_t = vpool.tile([P, H, D + 1], ADT, tag=f"vs{tt}")
                    MEMSET_ENG(v_t[:, :, D : D + 1], 1.0)
                    pss = [ps_mm.tile([P, S], F32, tag="mm", name=f"psv{c}")
                           for c in range(CH)]
                    for k in range(KT):
                        for ch in range(CH):
                            nc.tensor.matmul(
                                pss[ch][:, :CHW],
                                xts[:, k, tt * P : (tt + 1) * P],
                                w_sb["wv"][:, k, ch * CHW : (ch + 1) * CHW],
                                start=(k == 0),
                                stop=(k == KT - 1),
                            )
                    for ch in range(CH):
                        psc = pss[ch][:, :CHW]
                        hpc = CHW // D  # heads per chunk (6)
                        dst = v_t[:, ch * hpc : (ch + 1) * hpc, 0:D]
                        VCP(out=dst, in_=psc.rearrange("p (h d) -> p h d", d=D))
                    if with_bias:
                        nc.vector.tensor_add(
                            out=v_t[:],
                            in0=v_t[:],
                            in1=bvb[:].rearrange("p (h d) -> p h d", d=D + 1),
                        )
                    vs.append(v_t)
                return vs

            def qk_scores(b, xts, xts8, hp):
                # Q^T / K^T for this head pair (feature tile hp)
                qk = {}
                for name, tag in (("wq", "qt"), ("wk", "kt")):
                    if sc_fp8:
                        dst = qkpool.tile([P, 2, S], FP8, tag=tag)
                    else:
                        dst = qkpool.tile([P, S], SCDT, tag=tag)
                    ps = ps_mm.tile([P, S], F32, tag="mm")
                    if qk_fp8:
                        # fp8 DoubleRow: two 128-row k-subtiles per pass
                        for k in range(0, KT, 2):
                            nc.tensor.matmul(
                                ps[:],
                                w8_sb[name][:, k : k + 2, hp * P : (hp + 1) * P],
                                xts8[:, k : k + 2, :],
                                start=(k == 0),
                                stop=(k == KT - 2),
                                perf_mode=mybir.MatmulPerfMode.DoubleRow,
                            )
                    else:
                        for k in range(KT):
                            nc.tensor.matmul(
                                ps[:],
                                w_sb[name][:, k, hp * P : (hp + 1) * P],
                                xts[:, k, :],
                                start=(k == 0),
                                stop=(k == KT - 1),
                            )
                    if with_bias:
                        col = (0 if name == "wq" else KT) + hp
                        nc.vector.tensor_scalar_add(
                            dst[:, 0, :] if sc_fp8 else dst[:], ps[:],
                            bqk[:, col : col + 1],
                        )
                    else:
                        QKCP(out=dst[:, 0, :] if sc_fp8 else dst[:], in_=ps[:])
                    qk[tag] = dst
                qt, kt = qk["qt"], qk["kt"]

                # scores^T + exp, causal-trimmed per k-tile.  With
                # SC_SPLIT0 the i=0 (nq=512) tiles go through ps_mm per head,
                # so the serial scores->exp chain through the single ps_sc
                # bank loses its heaviest link and the two pools alternate.
                pts = []  # pts[i] = exp(scores^T) [P, 2, Nq] (heads of pair)
                for i in range(ST):
                    nq = S - i * P
                    qoff = i * P
                    if sc_split0 and i == 0:
                        pt = ppool.tile([P, 2, S], ADT, tag="pt")
                        for hh in range(2):
                            ro = hh * D
                            psh = ps_mm.tile([P, S], F32, tag="mm")
                            nc.tensor.matmul(
                                psh[:, 0:nq],
                                kt[ro : ro + D, i * P : (i + 1) * P],
                                qt[ro : ro + D, qoff:S],
                                start=True,
                                stop=True,
                                tile_position=(ro, 0),
                            )
                            nc.scalar.activation(
                                pt[:, hh, 0:nq],
                                psh[:, 0:nq],
                                mybir.ActivationFunctionType.Exp,
                                scale=0.125,
                            )
                    elif sc_perhead:
                        pt = ppool.tile([P, 2, S], ADT, tag="pt")
                        for hh in range(2):
                            ro = hh * D
                            psh = ps_sc.tile([P, S], F32, tag="sc")
                            nc.tensor.matmul(
                                psh[:, 0:nq],
                                kt[ro : ro + D, i * P : (i + 1) * P],
                                qt[ro : ro + D, qoff:S],
                                start=True,
                                stop=True,
                                tile_position=(ro, 0),
                            )
                            nc.scalar.activation(
                                pt[:, hh, 0:nq],
                                psh[:, 0:nq],
                                mybir.ActivationFunctionType.Exp,
                                scale=0.125,
                            )
                    else:
                        ps = ps_sc.tile(
                            [P, 2, 3 * P] if sc_split0 else [P, 2, S],
                            F32, tag="sc",
                        )
                        for hh in range(2):
                            ro = hh * D
                            if sc_fp8:
                                nc.tensor.matmul(
                                    ps[:, hh, 0:nq],
                                    kt[ro : ro + D, :, i * P : (i + 1) * P],
                                    qt[ro : ro + D, :, qoff:S],
                                    start=True,
                                    stop=True,
                                    tile_position=(ro, 0),
                                    perf_mode=mybir.MatmulPerfMode.DoubleRow,
                                )
                            else:
                                nc.tensor.matmul(
                                    ps[:, hh, 0:nq],
                                    kt[ro : ro + D, i * P : (i + 1) * P],
                                    qt[ro : ro + D, qoff:S],
                                    start=True,
                                    stop=True,
                                    tile_position=(ro, 0),
                                )
                        pt = ppool.tile([P, 2, S], ADT, tag="pt")
                        nc.scalar.activation(
                            pt[:, :, 0:nq],
                            ps[:, :, 0:nq],
                            mybir.ActivationFunctionType.Exp,
                            scale=0.125,
                        )
                    # causal mask: zero the upper triangle of the diagonal
                    # block, off the PE->ACT critical path (Pool engine,
                    # post-exp; all-SBUF operands so GpSimd can run it)
                    md = mdpool.tile([P, 2, P], ADT, tag="md")
                    MASK_ENG(
                        out=md[:], in0=pt[:, :, 0:P],
                        in1=mask01[:, None, :].to_broadcast((P, 2, P)),
                    )
                    pts.append((pt, md))
                return pts

            TPOST = os.environ.get("TPOST", "0") == "1"

            def pv_j(hp, pts, vs, j):
                yst = ypool.tile([P, 2, D], YSTDT, tag="yst")
                pv = ps_pv.tile([P, 2, D + 1], F32, tag="pv")
                for hh in range(2):
                    h = 2 * hp + hh
                    order = ([j] + list(range(j))) if PV_DIAG_FIRST else range(j + 1)
                    for ii, i in enumerate(order):
                        pt, md = pts[i]
                        lhsT = (
                            md[:, hh, :]
                            if i == j
                            else pt[:, hh, (j - i) * P : (j - i + 1) * P]
                        )
                        nc.tensor.matmul(
                            pv[:, hh, :],
                            lhsT,
                            vs[i][:, h, :],
                            start=(ii == 0),
                            stop=(ii == j),
                        )
                # one packed reciprocal + one broadcast multiply per
                # (head-pair, q-tile) on DVE, replacing 4 ACT/DVE ops
                r = rpool.tile([P, 2], F32, tag="r")
                nc.vector.reciprocal(r[:], pv[:, :, D])
                nc.vector.tensor_mul(
                    out=yst[:],
                    in0=pv[:, :, 0:D],
                    in1=r[:, :, None].to_broadcast((P, 2, D)),
                )
                return yst

            def yst_out(hp, yt, j, yst):
                yt_ps = ps_yt.tile([P, P], YSTDT, tag="ytp")
                nc.tensor.transpose(yt_ps[:], yst[:], ident)
                YCP(out=yt[:, hp, j * P : (j + 1) * P], in_=yt_ps[:])

            def pv_block(hp, pts, vs, yt):
                # P @ V_aug accumulated over k-tiles, then normalize,
                # then transpose Y back to feature-major.  With TPOST the
                # transposes of a j-pair are deferred until after both PV
                # chains so they do not head-of-line-block the PE queue
                # while the DVE normalize completes.
                if TPOST:
                    for jp in range(0, ST, 2):
                        ysts = [(j, pv_j(hp, pts, vs, j)) for j in (jp, jp + 1)]
                        for j, yst in ysts:
                            yst_out(hp, yt, j, yst)
                else:
                    for j in range(ST):
                        yst = pv_j(hp, pts, vs, j)
                        yst_out(hp, yt, j, yst)

            O_DMA = os.environ.get("O_DMA", "0") == "1" and not with_bias

            def oproj_tt(b, yt, tt):
                tok0 = (b % B_CORE) * S
                # k outer / ch inner: both ch matmuls share the stationary
                # (yt k-slice) so half the InstLdweights are elided.
                pss = [ps_mm.tile([P, S], F32, tag="mm", name=f"pso{c}")
                       for c in range(CH)]
                for k in range(KT):
                    for ch in range(CH):
                        nc.tensor.matmul(
                            pss[ch][:, :CHW],
                            yt[:, k, tt * P : (tt + 1) * P],
                            w_sb["wo"][:, k, ch * CHW : (ch + 1) * CHW],
                            start=(k == 0),
                            stop=(k == KT - 1),
                        )
                if O_DMA:
                    # DMA y straight out of PSUM, skipping the SBUF bounce
                    for ch in range(CH):
                        nc.sync.dma_start(
                            y_d[
                                tok0 + tt * P : tok0 + (tt + 1) * P,
                                ch * CHW : (ch + 1) * CHW,
                            ],
                            pss[ch][:, :CHW],
                        )
                else:
                    o_sb = opool.tile([P, E], F32, tag="osb")
                    for ch in range(CH):
                        OCP(out=o_sb[:, ch * CHW : (ch + 1) * CHW], in_=pss[ch][:, :CHW])
                    if with_bias:
                        nc.vector.tensor_add(out=o_sb[:], in0=o_sb[:], in1=bob[:])
                    nc.sync.dma_start(
                        y_d[tok0 + tt * P : tok0 + (tt + 1) * P, :], o_sb[:]
                    )

            def run_batches(batches):
                # Software-pipelined emission: scores of head-pair hp+1 are
                # emitted before the PV block of hp, so the tensor engine's
                # in-order stream always has matmuls to run while the
                # mask(DVE) -> exp(ACT) -> normalize(DVE) chains drain.
                load(0, batches[0])
                pending_o = None  # (b, yt) of the previous batch
                for idx, b in enumerate(batches):
                    xts, xts8 = xts_t.pop(idx)
                    vs = vproj(b, xts)
                    yt = ytpool.tile([P, KT, S], YTDT, tag="yt")
                    pts_next = qk_scores(b, xts, xts8, 0)
                    for hp in range(HP):
                        pts_cur = pts_next
                        # previous batch's output projection, one token tile
                        # at a time, spread through the PV chain gaps
                        if pending_o is not None and hp < ST:
                            oproj_tt(*pending_o, hp)
                        if hp == 2 and idx + 1 < len(batches):
                            load(idx + 1, batches[idx + 1])
                        if hp + 1 < HP:
                            pts_next = qk_scores(b, xts, xts8, hp + 1)
                        pv_block(hp, pts_cur, vs, yt)
                    pending_o = (b, yt)
                for tt in range(ST):
                    oproj_tt(*pending_o, tt)

            # hw_loop body covers `unroll` logical iterations to amortize the
            # For_i boundary sync; repeat must be a multiple of unroll.
            unroll = int(os.environ.get("LOOP_UNROLL", "2"))
            if hw_loop and repeat > 1:
                if repeat % unroll != 0:
                    unroll = 1
                body = [b % B_CORE for b in range(B_CORE * unroll)]
                with tc.For_i(0, repeat // unroll, 1):
                    run_batches(body)
            else:
                run_batches([b % B_CORE for b in range(B_CORE * repeat)])

    nc.compile()
    return nc


def _host_consts():
    ident = np.eye(P, dtype=np.float32)
    k_idx = np.arange(P, dtype=np.int64)[:, None]
    q_idx = np.arange(P, dtype=np.int64)[None, :]
    maskb = np.where(k_idx <= q_idx, 0.0, NEG).astype(np.float32)
    mask01 = (k_idx <= q_idx).astype(np.float32)
    return np.concatenate([ident, maskb, mask01], axis=1)  # [P, 3P]


def _host_consts_bf16():
    import ml_dtypes

    ident = np.eye(P, dtype=np.float32)
    k_idx = np.arange(P, dtype=np.int64)[:, None]
    q_idx = np.arange(P, dtype=np.int64)[None, :]
    mask01 = (k_idx <= q_idx).astype(np.float32)
    return np.concatenate([ident, mask01], axis=1).astype(ml_dtypes.bfloat16)


_PROG_CACHE = {}


# fp32r (relaxed single-pass fp32 matmul, ~2e-4 rel err, 4x PE throughput) is
# used by default; set BASS_MM_F32=1 for strict fp32 matmuls (~2x slower).
USE_F32R = os.environ.get("BASS_MM_F32", "0") != "1"
# fp8e4m3 DoubleRow Q/K projections (2 k-subtiles per PE pass).
USE_QK_FP8 = os.environ.get("QK_FP8", "1") == "1"


def _get_program(with_bias: bool):
    if with_bias not in _PROG_CACHE:
        _PROG_CACHE[with_bias] = build_program(
            with_bias, r_proj=USE_F32R, r_scores=USE_F32R
        )
    return _PROG_CACHE[with_bias]


def make_in_maps(x, Wq, bq, Wk, bk, Wv, bv, Wo, bo, with_bias, att_bf16=True):
    import ml_dtypes

    consts = _host_consts()
    lowinst = os.environ.get("LOW_INST", "0") == "1" and att_bf16
    wo_dt = (np.float32 if lowinst else ml_dtypes.bfloat16) if att_bf16 else np.float32
    xv_bf16 = os.environ.get("XV_BF16", "1") == "1" and att_bf16
    x_dt = ml_dtypes.bfloat16 if xv_bf16 else np.float32
    wv_dt = ml_dtypes.bfloat16 if xv_bf16 else np.float32
    maps = []
    for c in range(N_CORES):
        xc = np.ascontiguousarray(
            x[c * B_CORE : (c + 1) * B_CORE]  # [B_CORE, S, E]
            .reshape(TOK, E)
            .T  # [E, TOK]
        ).astype(np.float32)
        m = {
            "xt": np.ascontiguousarray(xc.astype(x_dt)),
            "wv": np.ascontiguousarray(np.asarray(Wv, np.float32).astype(wv_dt)),
            "wo": np.ascontiguousarray(np.asarray(Wo).astype(wo_dt)),
            "consts": consts,
        }
        if not USE_QK_FP8:
            m["wq"] = np.ascontiguousarray(Wq, dtype=np.float32)
            m["wk"] = np.ascontiguousarray(Wk, dtype=np.float32)
        if att_bf16:
            m["cb"] = _host_consts_bf16()
        if USE_QK_FP8:
            f8 = ml_dtypes.float8_e4m3
            m["xt8"] = np.ascontiguousarray(xc.astype(f8))
            m["wq8"] = np.ascontiguousarray(np.asarray(Wq, np.float32).astype(f8))
            m["wk8"] = np.ascontiguousarray(np.asarray(Wk, np.float32).astype(f8))
        if with_bias:
            bqk = np.concatenate(
                [np.asarray(bq).reshape(KT, P).T, np.asarray(bk).reshape(KT, P).T],
                axis=1,
            ).astype(np.float32)
            bvb = np.zeros((P, H, D + 1), np.float32)
            bvb[:, :, :D] = np.broadcast_to(np.asarray(bv).reshape(H, D), (P, H, D))
            m["bqk"] = np.ascontiguousarray(bqk)
            m["bvb"] = np.ascontiguousarray(bvb.reshape(P, H * (D + 1)))
            m["bob"] = np.ascontiguousarray(
                np.broadcast_to(np.asarray(bo, dtype=np.float32), (P, E))
            )
        maps.append(m)
    return maps


def kernel(x, Wq, bq, Wk, bk, Wv, bv, Wo, bo):
    from concourse.bass_utils import run_bass_kernel_spmd

    x = np.asarray(x, dtype=np.float32)
    with_bias = any(
        float(np.abs(np.asarray(b)).max()) != 0.0 for b in (bq, bk, bv, bo)
    )
    nc = _get_program(with_bias)
    in_maps = make_in_maps(x, Wq, bq, Wk, bk, Wv, bv, Wo, bo, with_bias)
    res = run_bass_kernel_spmd(nc, in_maps, core_ids=list(range(N_CORES)))
    out = np.empty((B_FULL, S, E), dtype=np.float32)
    for c in range(N_CORES):
        out[c * B_CORE : (c + 1) * B_CORE] = res.results[c]["y"].reshape(B_CORE, S, E)
    return out



# revision 18
# speedup vs baseline: 1.1177x; 1.0035x over previous
"""Multi-head causal self-attention (B=32, S=512, E=768, H=12, D=64) on 8 TRN2 cores.

Sharding: pure data-parallel over batch (4 batches per core), no collectives.

Per-core layout strategy:
  - x is fed pre-transposed (feature-major) as xT [E, 2048tok].
  - Q^T, K^T are computed feature-major per head-pair (feature tile == head
    pair):  QT_hp = Wq[:, hp].T @ xT   (lhsT=Wq slice, rhs=xT)
  - V is computed token-major with an extra all-ones column per head
    ("V_aug" [tok, H*(D+1)]); the ones column makes the P@V matmul also
    produce the softmax denominators.
  - scores^T[k,q] = K Q^T computed per (head, k-tile of 128 tokens) with the
    causal-trimmed q range [128*i, 512), both heads of a pair packed into the
    128x128 PE array via tile_position row groups.
  - exp() on ScalarE reads score PSUM directly (1/sqrt(D) folded into exp's
    scale), both heads in one call; the causal mask is a post-exp 0/1
    multiply of just the diagonal 128x128 block on VectorE, kept OFF the
    PE->ACT critical path.
  - P@V: out[q, D+1] accumulated over k-tiles i<=j in PSUM; reciprocal of
    column D (the ones-column sum = softmax denominator) normalizes via a
    ScalarE copy with per-partition scale.
  - Y (token-major) is transposed 128x128 via TensorE back to feature-major
    for the output projection, which lands token-major for a contiguous DMA.
  - Emission is software-pipelined (scores of head-pair hp+1 before the PV
    block of hp; next batch's xT DMA prefetched mid-batch) so the in-order
    engine streams always have independent matmuls to hide the cross-engine
    softmax chains.

Dtype strategy (PE cost = moving-dim size x cycles/row; fp32=4, fp32r=1 only
when moving>=256, bf16=1 always, fp8e4+DoubleRow=0.5):
  - V / O projections: fp32r operands (moving dims 384 -> already 1 cyc/row).
  - Q/K projections: fp8e4m3 operands with MatmulPerfMode.DoubleRow, feeding
    two 128-row k-subtiles per PE pass ([P, 2, *] slices of the [P, KT, *]
    layout).  Dominant error source: ~1.2e-2 absmax-rel end to end (gate is
    2e-2).  QK_FP8=0 falls back to fp32r (error ~2e-3).
  - Attention path (Q^T/K^T tiles, exp output P, V tiles, Y, Wo): bf16.
    This makes every PV matmul (free=65) and nq=128 score tile 1 cyc/row.
  - exp() reads f32 score PSUM, emits bf16; softmax normalization is a packed
    per-head-pair reciprocal + broadcast multiply on DVE.
  - hw_loop timing programs wrap a LOOP_UNROLL (default 2) iteration body in
    For_i to amortize the ~32us loop-boundary sync.  LOOP_UNROLL=4 measured
    +8us/iter on HW (bigger body hurts more than the halved barrier helps;
    likely instruction-fetch locality), so 2 stays the default.
  - V / O projections emit k-outer / ch-inner so consecutive matmuls
    alternate between the two ch PSUM banks (same stationary back to back):
    measured -6.6us/iter on HW vs the ch-outer ordering.
  - SC_PERHEAD=1 (per-head score tiles, 2 PSUM banks) looks -2us in
    TimelineSim but measured +20us/iter on HW: keep OFF.
  - XV_BF16=1 (default ON): x and Wv in bf16 halve the per-iteration xt
    DMA (6.3 -> 3.1 MB).  Measured -7.1us/iter on HW: the 8 cores share
    HBM bandwidth, so DMA volume matters more than single-core sim says.
  - LOOP_STAGGER=1 + XPIPE=1 (default ON): staggered For_i semaphore
    reset instead of the all-engine barrier, and a cross-trip x prefetch
    (first batch's x tiles peeled before the loop; the body re-DMAs the
    same ring slot mid-trip for the next trip).  Both verified correct on
    the timed hw_loop program; ~-1us/iter each, within run noise.
  - Y_BF16=1 (bf16 y DMA) measured no gain (y writeback already hidden):
    left OFF to keep its ~3e-3 error headroom.  O_DMA=1 fails an internal
    assert at build.  The remaining gap to sim (~35us) is cross-engine
    chain stalls (sim: 31.6us PE idle waiting on qt/kt/pt/md/yt tiles)
    plus ACT exp occupancy (69us/iter) -- a score->exp->PV chain
    restructure is the next real lever, not loop/DMA knobs.
  - fp8 V or O projections are numerically dead: host-sim absmax-rel 4e-2
    vs the 2e-2 gate (vs 1.2e-2 for the current QK-fp8-only config).
Set BASS_MM_F32=1 + QK_FP8=0 for a strict-fp32 fallback.
"""

import os
import sys

import numpy as np

for _p in ("/opt/trn_rl_repo", "/opt/trn_rl_repo/concourse"):
    if _p not in sys.path:
        sys.path.insert(0, _p)

import concourse.bass as bass
import concourse.bacc as bacc
import concourse.mybir as mybir
import concourse.tile as tile

P = 128
E = 768
S = 512
H = 12
D = 64
HP = H // 2          # head pairs
KT = E // P          # 6 feature k-tiles
N_CORES = 8
B_FULL = 32
B_CORE = B_FULL // N_CORES   # 4 batches per core
TOK = B_CORE * S             # 2048 tokens per core
ST = S // P                  # 4 token tiles per sequence
NEG = -1.0e6                 # pre-scale mask bias; exp(0.125 * -1e6) == 0
F32 = mybir.dt.float32

# number of 384-wide chunks for the V / O projections
CH = 2
CHW = E // CH  # 384


def build_program(with_bias: bool, repeat: int = 1, hw_loop: bool = False,
                  r_proj: bool = False, r_scores: bool = False, phases: int = 3,
                  att_bf16: bool = True, qk_fp8: bool | None = None):
    if qk_fp8 is None:
        qk_fp8 = USE_QK_FP8
    PDT = mybir.dt.float32r if r_proj else F32   # proj operands (x, weights)
    BF16 = mybir.dt.bfloat16
    FP8 = mybir.dt.float8e4
    # attention-path operand dtype: qt/kt (scores), pt/md (probs), vs (values),
    # yst/yt (attention out) and wo.  bf16 gets 1 PE cycle/row on ALL matmul
    # shapes (fp32 is 4; fp32r is 4 whenever the moving dim < 256, which hits
    # every PV matmul [free=65] and the nq=128 score tiles).
    ADT = BF16 if att_bf16 else (mybir.dt.float32r if r_scores else F32)
    # fp32r for scores / transpose / oproj: f32(r) matmuls self-load their
    # weights (no separate InstLdweights), cutting ~480 PE instructions per
    # iteration.  Measured +10us on HW (= its exec-cycle cost): the PE is
    # exec-cycle bound, not dispatch bound, so this stays OFF.
    lowinst = os.environ.get("LOW_INST", "0") == "1" and att_bf16
    # fp8 DoubleRow scores: qt/kt stored [P, 2, S] e4m3 with subtile 1
    # pre-zeroed; numerically correct on HW but measured +20us (the doubled
    # moving operand streams at full length), so this stays OFF.
    sc_fp8 = (os.environ.get("SC_FP8", "0") == "1") and qk_fp8 and not lowinst
    # route i=0 score tiles through ps_mm to break the ps_sc serial chain
    sc_split0 = os.environ.get("SC_SPLIT0", "0") == "1" and not sc_fp8
    # per-head [P,S] score tiles, bufs=2 in the same 2 PSUM banks: the two
    # head chains alternate banks, halving the serial scores->exp backbone
    sc_perhead = os.environ.get("SC_PERHEAD", "0") == "1" and not sc_fp8 and not sc_split0
    SCDT = mybir.dt.float32r if lowinst else ADT   # qt/kt (scores operands)
    YSTDT = F32 if lowinst else ADT                # normalize out / transpose in
    YTDT = mybir.dt.float32r if lowinst else ADT   # yt (oproj stationary)
    WODT = (mybir.dt.float32r if lowinst else BF16) if att_bf16 else PDT
    # bf16 x + Wv: halves the per-iteration xt DMA; vproj stays 1 cyc/row.
    xv_bf16 = os.environ.get("XV_BF16", "1") == "1" and att_bf16
    XDT = BF16 if xv_bf16 else PDT
    WVDT = BF16 if xv_bf16 else PDT
    nc = bacc.Bacc(None)
    _eng = {"dve": nc.vector, "act": nc.scalar, "pool": nc.gpsimd, "any": nc.any}
    MASK_ENG = _eng[os.environ.get("MASK_ENG", "dve")].tensor_mul
    MEMSET_ENG = _eng[os.environ.get("MEMSET_ENG", "pool")].memset
    QKCP = _eng[os.environ.get("QKCP_ENG", "any")].tensor_copy
    VCP = _eng[os.environ.get("VCP_ENG", "any")].tensor_copy
    YCP = _eng[os.environ.get("YCP_ENG", "any")].tensor_copy
    OCP = _eng[os.environ.get("OCP_ENG", "any")].tensor_copy
    PV_DIAG_FIRST = os.environ.get("PV_DIAG_FIRST", "0") == "1"

    xt_d = nc.dram_tensor("xt", [E, TOK], XDT, kind="ExternalInput")
    _wnames = ("wv", "wo") if qk_fp8 else ("wq", "wk", "wv", "wo")
    _wdt = {"wq": PDT, "wk": PDT, "wv": WVDT, "wo": WODT}
    w_d = {
        n: nc.dram_tensor(n, [E, E], _wdt[n], kind="ExternalInput")
        for n in _wnames
    }
    consts_d = nc.dram_tensor("consts", [P, 3 * P], F32, kind="ExternalInput")
    if att_bf16:
        # bf16 identity (PE transpose moving operand) + bf16 causal 0/1 mask
        cb_d = nc.dram_tensor("cb", [P, 2 * P], BF16, kind="ExternalInput")
    if qk_fp8:
        xt8_d = nc.dram_tensor("xt8", [E, TOK], FP8, kind="ExternalInput")
        w8_d = {
            n: nc.dram_tensor(n + "8", [E, E], FP8, kind="ExternalInput")
            for n in ("wq", "wk")
        }
    if with_bias:
        bqk_d = nc.dram_tensor("bqk", [P, 2 * KT], F32, kind="ExternalInput")
        bv_d = nc.dram_tensor("bvb", [P, H * (D + 1)], F32, kind="ExternalInput")
        bo_d = nc.dram_tensor("bob", [P, E], F32, kind="ExternalInput")
    # bf16 y output: halves the y DMA (6.3 -> 3.1 MB per iteration); host
    # converts back to f32.  Adds <= ~0.2% per-element rounding on the output.
    y_bf16 = os.environ.get("Y_BF16", "0") == "1" and att_bf16 and not with_bias
    y_d = nc.dram_tensor("y", [TOK, E], BF16 if y_bf16 else F32,
                         kind="ExternalOutput")

    with tile.TileContext(nc) as tc:
        with (
            tc.tile_pool(name="wpool", bufs=1) as wpool,
            tc.tile_pool(name="xpool", bufs=2) as xpool,
            tc.tile_pool(name="qkpool", bufs=int(os.environ.get("B_QK", "3"))) as qkpool,
            tc.tile_pool(name="vpool", bufs=int(os.environ.get("B_VS", "2"))) as vpool,
            tc.tile_pool(name="ppool", bufs=int(os.environ.get("B_PT", "8"))) as ppool,
            tc.tile_pool(name="mdpool", bufs=int(os.environ.get("B_MD", "8"))) as mdpool,
            tc.tile_pool(name="ypool", bufs=4) as ypool,
            tc.tile_pool(name="ytpool", bufs=2) as ytpool,
            tc.tile_pool(name="opool", bufs=2) as opool,
            tc.tile_pool(name="rpool", bufs=4) as rpool,
            tc.tile_pool(name="ps_mm", bufs=int(os.environ.get("B_MM", "3")), space="PSUM") as ps_mm,
            tc.tile_pool(name="ps_sc", bufs=int(os.environ.get("B_SC", "2" if (os.environ.get("SC_PERHEAD", "0") == "1") else "1")), space="PSUM") as ps_sc,
            tc.tile_pool(name="ps_pv", bufs=int(os.environ.get("B_PV", "2")), space="PSUM") as ps_pv,
            tc.tile_pool(name="ps_yt", bufs=int(os.environ.get("B_YT", "1")), space="PSUM") as ps_yt,
        ):
            # ---- persistent constants ----
            w_sb = {}
            for n in _wnames:
                t = wpool.tile([P, KT, E], _wdt[n], tag=n)
                nc.sync.dma_start(t[:], w_d[n][:].rearrange("(ko ki) m -> ki ko m", ki=P))
                w_sb[n] = t
            cons = wpool.tile([P, 3 * P], F32, tag="consts")  # masks stay f32
            nc.sync.dma_start(cons[:], consts_d[:])
            ident = cons[:, 0:P]
            mask01 = cons[:, 2 * P : 3 * P]
            if att_bf16:
                cb = wpool.tile([P, 2 * P], mybir.dt.bfloat16, tag="cb")
                nc.sync.dma_start(cb[:], cb_d[:])
                if not lowinst:
                    ident = cb[:, 0:P]
                mask01 = cb[:, P : 2 * P]
            w8_sb = {}
            if qk_fp8:
                for n in ("wq", "wk"):
                    t = wpool.tile([P, KT, E], FP8, tag=n + "8")
                    nc.sync.dma_start(
                        t[:], w8_d[n][:].rearrange("(ko ki) m -> ki ko m", ki=P)
                    )
                    w8_sb[n] = t
            if with_bias:
                bqk = wpool.tile([P, 2 * KT], F32, tag="bqk")
                nc.sync.dma_start(bqk[:], bqk_d[:])
                bvb = wpool.tile([P, H * (D + 1)], F32, tag="bvb")
                nc.sync.dma_start(bvb[:], bv_d[:])
                bob = wpool.tile([P, E], F32, tag="bob")
                nc.sync.dma_start(bob[:], bo_d[:])

            if sc_fp8:
                # pre-zero subtile 1 of every qk pool buffer once; the live
                # copies only ever write subtile 0, so these zeros persist
                for _ in range(int(os.environ.get("B_QK", "3"))):
                    for tag in ("qt", "kt"):
                        tz = qkpool.tile([P, 2, S], FP8, tag=tag, name="tz")
                        MEMSET_ENG(tz[:, 1, :], 0.0)

            xt_r = xt_d[:].rearrange("(ko ki) t -> ki ko t", ki=P)
            if qk_fp8:
                xt8_r = xt8_d[:].rearrange("(ko ki) t -> ki ko t", ki=P)

            xts_t = {}

            def load(pos, b):
                tok0 = (b % B_CORE) * S
                xts = xpool.tile([P, KT, S], XDT, tag="xts")
                nc.sync.dma_start(xts[:], xt_r[:, :, tok0 : tok0 + S])
                xts8 = None
                if qk_fp8:
                    xts8 = xpool.tile([P, KT, S], FP8, tag="xts8")
                    nc.sync.dma_start(xts8[:], xt8_r[:, :, tok0 : tok0 + S])
                xts_t[pos] = (xts, xts8)

            def vproj(b, xts):
                # ---- V projection (token-major, augmented with ones cols) ----
                # k outer / ch inner: the two ch matmuls share the same
                # stationary (xts k-slice), so legalization skips every other
                # InstLdweights (48 -> 24 weight loads per batch).
                vs = []
                for tt in range(ST):
                    v_t = vpool.tile([P, H, D + 1], ADT, tag=f"vs{tt}")
                    MEMSET_ENG(v_t[:, :, D : D + 1], 1.0)
                    pss = [ps_mm.tile([P, S], F32, tag="mm", name=f"psv{c}")
                           for c in range(CH)]
                    for k in range(KT):
                        for ch in range(CH):
                            nc.tensor.matmul(
                                pss[ch][:, :CHW],
                                xts[:, k, tt * P : (tt + 1) * P],
                                w_sb["wv"][:, k, ch * CHW : (ch + 1) * CHW],
                                start=(k == 0),
                                stop=(k == KT - 1),
                            )
                    for ch in range(CH):
                        psc = pss[ch][:, :CHW]
                        hpc = CHW // D  # heads per chunk (6)
                        dst = v_t[:, ch * hpc : (ch + 1) * hpc, 0:D]
                        VCP(out=dst, in_=psc.rearrange("p (h d) -> p h d", d=D))
                    if with_bias:
                        nc.vector.tensor_add(
                            out=v_t[:],
                            in0=v_t[:],
                            in1=bvb[:].rearrange("p (h d) -> p h d", d=D + 1),
                        )
                    vs.append(v_t)
                return vs

            def qk_scores(b, xts, xts8, hp):
                # Q^T / K^T for this head pair (feature tile hp)
                qk = {}
                for name, tag in (("wq", "qt"), ("wk", "kt")):
                    if sc_fp8:
                        dst = qkpool.tile([P, 2, S], FP8, tag=tag)
                    else:
                        dst = qkpool.tile([P, S], SCDT, tag=tag)
                    ps = ps_mm.tile([P, S], F32, tag="mm")
                    if qk_fp8:
                        # fp8 DoubleRow: two 128-row k-subtiles per pass
                        for k in range(0, KT, 2):
                            nc.tensor.matmul(
                                ps[:],
                                w8_sb[name][:, k : k + 2, hp * P : (hp + 1) * P],
                                xts8[:, k : k + 2, :],
                                start=(k == 0),
                                stop=(k == KT - 2),
                                perf_mode=mybir.MatmulPerfMode.DoubleRow,
                            )
                    else:
                        for k in range(KT):
                            nc.tensor.matmul(
                                ps[:],
                                w_sb[name][:, k, hp * P : (hp + 1) * P],
                                xts[:, k, :],
                                start=(k == 0),
                                stop=(k == KT - 1),
                            )
                    if with_bias:
                        col = (0 if name == "wq" else KT) + hp
                        nc.vector.tensor_scalar_add(
                            dst[:, 0, :] if sc_fp8 else dst[:], ps[:],
                            bqk[:, col : col + 1],
                        )
                    else:
                        QKCP(out=dst[:, 0, :] if sc_fp8 else dst[:], in_=ps[:])
                    qk[tag] = dst
                qt, kt = qk["qt"], qk["kt"]

                # scores^T + exp, causal-trimmed per k-tile.  With
                # SC_SPLIT0 the i=0 (nq=512) tiles go through ps_mm per head,
                # so the serial scores->exp chain through the single ps_sc
                # bank loses its heaviest link and the two pools alternate.
                pts = []  # pts[i] = exp(scores^T) [P, 2, Nq] (heads of pair)
                for i in range(ST):
                    nq = S - i * P
                    qoff = i * P
                    if sc_split0 and i == 0:
                        pt = ppool.tile([P, 2, S], ADT, tag="pt")
                        for hh in range(2):
                            ro = hh * D
                            psh = ps_mm.tile([P, S], F32, tag="mm")
                            nc.tensor.matmul(
                                psh[:, 0:nq],
                                kt[ro : ro + D, i * P : (i + 1) * P],
                                qt[ro : ro + D, qoff:S],
                                start=True,
                                stop=True,
                                tile_position=(ro, 0),
                            )
                            nc.scalar.activation(
                                pt[:, hh, 0:nq],
                                psh[:, 0:nq],
                                mybir.ActivationFunctionType.Exp,
                                scale=0.125,
                            )
                    elif sc_perhead:
                        pt = ppool.tile([P, 2, S], ADT, tag="pt")
                        for hh in range(2):
                            ro = hh * D
                            psh = ps_sc.tile([P, S], F32, tag="sc")
                            nc.tensor.matmul(
                                psh[:, 0:nq],
                                kt[ro : ro + D, i * P : (i + 1) * P],
                                qt[ro : ro + D, qoff:S],
                                start=True,
                                stop=True,
                                tile_position=(ro, 0),
                            )
                            nc.scalar.activation(
                                pt[:, hh, 0:nq],
                                psh[:, 0:nq],
                                mybir.ActivationFunctionType.Exp,
                                scale=0.125,
                            )
                    else:
                        ps = ps_sc.tile(
                            [P, 2, 3 * P] if sc_split0 else [P, 2, S],
                            F32, tag="sc",
                        )
                        for hh in range(2):
                            ro = hh * D
                            if sc_fp8:
                                nc.tensor.matmul(
                                    ps[:, hh, 0:nq],
                                    kt[ro : ro + D, :, i * P : (i + 1) * P],
                                    qt[ro : ro + D, :, qoff:S],
                                    start=True,
                                    stop=True,
                                    tile_position=(ro, 0),
                                    perf_mode=mybir.MatmulPerfMode.DoubleRow,
                                )
                            else:
                                nc.tensor.matmul(
                                    ps[:, hh, 0:nq],
                                    kt[ro : ro + D, i * P : (i + 1) * P],
                                    qt[ro : ro + D, qoff:S],
                                    start=True,
                                    stop=True,
                                    tile_position=(ro, 0),
                                )
                        pt = ppool.tile([P, 2, S], ADT, tag="pt")
                        nc.scalar.activation(
                            pt[:, :, 0:nq],
                            ps[:, :, 0:nq],
                            mybir.ActivationFunctionType.Exp,
                            scale=0.125,
                        )
                    # causal mask: zero the upper triangle of the diagonal
                    # block, off the PE->ACT critical path (Pool engine,
                    # post-exp; all-SBUF operands so GpSimd can run it)
                    md = mdpool.tile([P, 2, P], ADT, tag="md")
                    MASK_ENG(
                        out=md[:], in0=pt[:, :, 0:P],
                        in1=mask01[:, None, :].to_broadcast((P, 2, P)),
                    )
                    pts.append((pt, md))
                return pts

            TPOST = os.environ.get("TPOST", "0") == "1"

            def pv_j(hp, pts, vs, j):
                yst = ypool.tile([P, 2, D], YSTDT, tag="yst")
                pv = ps_pv.tile([P, 2, D + 1], F32, tag="pv")
                for hh in range(2):
                    h = 2 * hp + hh
                    order = ([j] + list(range(j))) if PV_DIAG_FIRST else range(j + 1)
                    for ii, i in enumerate(order):
                        pt, md = pts[i]
                        lhsT = (
                            md[:, hh, :]
                            if i == j
                            else pt[:, hh, (j - i) * P : (j - i + 1) * P]
                        )
                        nc.tensor.matmul(
                            pv[:, hh, :],
                            lhsT,
                            vs[i][:, h, :],
                            start=(ii == 0),
                            stop=(ii == j),
                        )
                # one packed reciprocal + one broadcast multiply per
                # (head-pair, q-tile) on DVE, replacing 4 ACT/DVE ops
                r = rpool.tile([P, 2], F32, tag="r")
                nc.vector.reciprocal(r[:], pv[:, :, D])
                nc.vector.tensor_mul(
                    out=yst[:],
                    in0=pv[:, :, 0:D],
                    in1=r[:, :, None].to_broadcast((P, 2, D)),
                )
                return yst

            def yst_out(hp, yt, j, yst):
                yt_ps = ps_yt.tile([P, P], YSTDT, tag="ytp")
                nc.tensor.transpose(yt_ps[:], yst[:], ident)
                YCP(out=yt[:, hp, j * P : (j + 1) * P], in_=yt_ps[:])

            def pv_block(hp, pts, vs, yt):
                # P @ V_aug accumulated over k-tiles, then normalize,
                # then transpose Y back to feature-major.  With TPOST the
                # transposes of a j-pair are deferred until after both PV
                # chains so they do not head-of-line-block the PE queue
                # while the DVE normalize completes.
                if TPOST:
                    for jp in range(0, ST, 2):
                        ysts = [(j, pv_j(hp, pts, vs, j)) for j in (jp, jp + 1)]
                        for j, yst in ysts:
                            yst_out(hp, yt, j, yst)
                else:
                    for j in range(ST):
                        yst = pv_j(hp, pts, vs, j)
                        yst_out(hp, yt, j, yst)

            # O_DMA needs f32 y (DMA cannot convert PSUM f32 -> bf16)
            O_DMA = (os.environ.get("O_DMA", "0") == "1" and not with_bias
                     and not y_bf16)

            def oproj_tt(b, yt, tt):
                tok0 = (b % B_CORE) * S
                # k outer / ch inner: both ch matmuls share the stationary
                # (yt k-slice) so half the InstLdweights are elided.
                pss = [ps_mm.tile([P, S], F32, tag="mm", name=f"pso{c}")
                       for c in range(CH)]
                for k in range(KT):
                    for ch in range(CH):
                        nc.tensor.matmul(
                            pss[ch][:, :CHW],
                            yt[:, k, tt * P : (tt + 1) * P],
                            w_sb["wo"][:, k, ch * CHW : (ch + 1) * CHW],
                            start=(k == 0),
                            stop=(k == KT - 1),
                        )
                if O_DMA:
                    # DMA y straight out of PSUM, skipping the SBUF bounce
                    for ch in range(CH):
                        nc.sync.dma_start(
                            y_d[
                                tok0 + tt * P : tok0 + (tt + 1) * P,
                                ch * CHW : (ch + 1) * CHW,
                            ],
                            pss[ch][:, :CHW],
                        )
                else:
                    o_sb = opool.tile([P, E], BF16 if y_bf16 else F32, tag="osb")
                    for ch in range(CH):
                        OCP(out=o_sb[:, ch * CHW : (ch + 1) * CHW], in_=pss[ch][:, :CHW])
                    if with_bias:
                        nc.vector.tensor_add(out=o_sb[:], in0=o_sb[:], in1=bob[:])
                    nc.sync.dma_start(
                        y_d[tok0 + tt * P : tok0 + (tt + 1) * P, :], o_sb[:]
                    )

            def run_batches(batches, cross_trip=False):
                # Software-pipelined emission: scores of head-pair hp+1 are
                # emitted before the PV block of hp, so the tensor engine's
                # in-order stream always has matmuls to run while the
                # mask(DVE) -> exp(ACT) -> normalize(DVE) chains drain.
                # cross_trip: position 0's x tiles were loaded by the caller
                # (peel before For_i); the body re-loads them mid-body for
                # the NEXT trip so the post-barrier start never waits on DMA.
                if not cross_trip:
                    load(0, batches[0])
                pending_o = None  # (b, yt) of the previous batch
                for idx, b in enumerate(batches):
                    xts, xts8 = xts_t.pop(idx)
                    vs = vproj(b, xts)
                    yt = ytpool.tile([P, KT, S], YTDT, tag="yt")
                    pts_next = qk_scores(b, xts, xts8, 0)
                    for hp in range(HP):
                        pts_cur = pts_next
                        # previous batch's output projection, one token tile
                        # at a time, spread through the PV chain gaps
                        if pending_o is not None and hp < ST:
                            oproj_tt(*pending_o, hp)
                        if hp == 2:
                            if idx + 1 < len(batches):
                                load(idx + 1, batches[idx + 1])
                            elif cross_trip:
                                # next trip's first batch (same ring slot as
                                # the peel: allocation counts per tag are
                                # equal every trip, so the address matches)
                                load(0, batches[0])
                        if hp + 1 < HP:
                            pts_next = qk_scores(b, xts, xts8, hp + 1)
                        pv_block(hp, pts_cur, vs, yt)
                    pending_o = (b, yt)
                for tt in range(ST):
                    oproj_tt(*pending_o, tt)

            # hw_loop body covers `unroll` logical iterations to amortize the
            # For_i boundary sync; repeat must be a multiple of unroll.
            unroll = int(os.environ.get("LOOP_UNROLL", "2"))
            # staggered_reset replaces the per-trip all-engine barrier with
            # per-stage semaphore resets staggered through the body;
            # hint_engines adds branch-prefetch hints on the back edge.
            stagger = os.environ.get("LOOP_STAGGER", "1") == "1"
            hints = (list(mybir.ALL_ENGINES)
                     if os.environ.get("LOOP_HINTS", "0") == "1" else ())
            xpipe = os.environ.get("XPIPE", "1") == "1"
            if hw_loop and repeat > 1:
                if repeat % unroll != 0:
                    unroll = 1
                body = [b % B_CORE for b in range(B_CORE * unroll)]
                if xpipe:
                    # peel the first x load; the body reloads slot 0 mid-trip
                    # for the next trip so the post-barrier start is DMA-free
                    load(0, body[0])
                with tc.For_i(0, repeat // unroll, 1,
                              staggered_reset=stagger, hint_engines=hints):
                    run_batches(body, cross_trip=xpipe)
                if stagger:
                    tc.epilogue_barrier()
            else:
                run_batches([b % B_CORE for b in range(B_CORE * repeat)])

    nc.compile()
    return nc


def _host_consts():
    ident = np.eye(P, dtype=np.float32)
    k_idx = np.arange(P, dtype=np.int64)[:, None]
    q_idx = np.arange(P, dtype=np.int64)[None, :]
    maskb = np.where(k_idx <= q_idx, 0.0, NEG).astype(np.float32)
    mask01 = (k_idx <= q_idx).astype(np.float32)
    return np.concatenate([ident, maskb, mask01], axis=1)  # [P, 3P]


def _host_consts_bf16():
    import ml_dtypes

    ident = np.eye(P, dtype=np.float32)
    k_idx = np.arange(P, dtype=np.int64)[:, None]
    q_idx = np.arange(P, dtype=np.int64)[None, :]
    mask01 = (k_idx <= q_idx).astype(np.float32)
    return np.concatenate([ident, mask01], axis=1).astype(ml_dtypes.bfloat16)


_PROG_CACHE = {}


# fp32r (relaxed single-pass fp32 matmul, ~2e-4 rel err, 4x PE throughput) is
# used by default; set BASS_MM_F32=1 for strict fp32 matmuls (~2x slower).
USE_F32R = os.environ.get("BASS_MM_F32", "0") != "1"
# fp8e4m3 DoubleRow Q/K projections (2 k-subtiles per PE pass).
USE_QK_FP8 = os.environ.get("QK_FP8", "1") == "1"


def _get_program(with_bias: bool):
    if with_bias not in _PROG_CACHE:
        _PROG_CACHE[with_bias] = build_program(
            with_bias, r_proj=USE_F32R, r_scores=USE_F32R
        )
    return _PROG_CACHE[with_bias]


def make_in_maps(x, Wq, bq, Wk, bk, Wv, bv, Wo, bo, with_bias, att_bf16=True):
    import ml_dtypes

    consts = _host_consts()
    lowinst = os.environ.get("LOW_INST", "0") == "1" and att_bf16
    wo_dt = (np.float32 if lowinst else ml_dtypes.bfloat16) if att_bf16 else np.float32
    xv_bf16 = os.environ.get("XV_BF16", "1") == "1" and att_bf16
    x_dt = ml_dtypes.bfloat16 if xv_bf16 else np.float32
    wv_dt = ml_dtypes.bfloat16 if xv_bf16 else np.float32
    maps = []
    for c in range(N_CORES):
        xc = np.ascontiguousarray(
            x[c * B_CORE : (c + 1) * B_CORE]  # [B_CORE, S, E]
            .reshape(TOK, E)
            .T  # [E, TOK]
        ).astype(np.float32)
        m = {
            "xt": np.ascontiguousarray(xc.astype(x_dt)),
            "wv": np.ascontiguousarray(np.asarray(Wv, np.float32).astype(wv_dt)),
            "wo": np.ascontiguousarray(np.asarray(Wo).astype(wo_dt)),
            "consts": consts,
        }
        if not USE_QK_FP8:
            m["wq"] = np.ascontiguousarray(Wq, dtype=np.float32)
            m["wk"] = np.ascontiguousarray(Wk, dtype=np.float32)
        if att_bf16:
            m["cb"] = _host_consts_bf16()
        if USE_QK_FP8:
            f8 = ml_dtypes.float8_e4m3
            m["xt8"] = np.ascontiguousarray(xc.astype(f8))
            m["wq8"] = np.ascontiguousarray(np.asarray(Wq, np.float32).astype(f8))
            m["wk8"] = np.ascontiguousarray(np.asarray(Wk, np.float32).astype(f8))
        if with_bias:
            bqk = np.concatenate(
                [np.asarray(bq).reshape(KT, P).T, np.asarray(bk).reshape(KT, P).T],
                axis=1,
            ).astype(np.float32)
            bvb = np.zeros((P, H, D + 1), np.float32)
            bvb[:, :, :D] = np.broadcast_to(np.asarray(bv).reshape(H, D), (P, H, D))
            m["bqk"] = np.ascontiguousarray(bqk)
            m["bvb"] = np.ascontiguousarray(bvb.reshape(P, H * (D + 1)))
            m["bob"] = np.ascontiguousarray(
                np.broadcast_to(np.asarray(bo, dtype=np.float32), (P, E))
            )
        maps.append(m)
    return maps


def kernel(x, Wq, bq, Wk, bk, Wv, bv, Wo, bo):
    from concourse.bass_utils import run_bass_kernel_spmd

    x = np.asarray(x, dtype=np.float32)
    with_bias = any(
        float(np.abs(np.asarray(b)).max()) != 0.0 for b in (bq, bk, bv, bo)
    )
    nc = _get_program(with_bias)
    in_maps = make_in_maps(x, Wq, bq, Wk, bk, Wv, bv, Wo, bo, with_bias)
    res = run_bass_kernel_spmd(nc, in_maps, core_ids=list(range(N_CORES)))
    out = np.empty((B_FULL, S, E), dtype=np.float32)
    for c in range(N_CORES):
        yc = np.asarray(res.results[c]["y"], dtype=np.float32)  # may be bf16
        out[c * B_CORE : (c + 1) * B_CORE] = yc.reshape(B_CORE, S, E)
    return out



# revision 33
# speedup vs baseline: 1.1313x; 1.0122x over previous
"""Multi-head causal self-attention (B=32, S=512, E=768, H=12, D=64) on 8 TRN2 cores.

Sharding: pure data-parallel over batch (4 batches per core), no collectives.

Per-core layout strategy:
  - x is fed pre-transposed (feature-major) as xT [E, 2048tok].
  - Q^T, K^T are computed feature-major per head-pair (feature tile == head
    pair):  QT_hp = Wq[:, hp].T @ xT   (lhsT=Wq slice, rhs=xT)
  - V is computed token-major with an extra all-ones column per head
    ("V_aug" [tok, H*(D+1)]); the ones column makes the P@V matmul also
    produce the softmax denominators.
  - scores^T[k,q] = K Q^T computed per (head, k-tile of 128 tokens) with the
    causal-trimmed q range [128*i, 512), both heads of a pair packed into the
    128x128 PE array via tile_position row groups.
  - exp() on ScalarE reads score PSUM directly (1/sqrt(D) folded into exp's
    scale), both heads in one call; the causal mask is a post-exp 0/1
    multiply of just the diagonal 128x128 block on VectorE, kept OFF the
    PE->ACT critical path.
  - P@V: out[q, D+1] accumulated over k-tiles i<=j in PSUM; reciprocal of
    column D (the ones-column sum = softmax denominator) normalizes via a
    ScalarE copy with per-partition scale.
  - Y (token-major) is transposed 128x128 via TensorE back to feature-major
    for the output projection, which lands token-major for a contiguous DMA.
  - Emission is software-pipelined (scores of head-pair hp+1 before the PV
    block of hp; next batch's xT DMA prefetched mid-batch) so the in-order
    engine streams always have independent matmuls to hide the cross-engine
    softmax chains.

Dtype strategy (PE cost = moving-dim size x cycles/row; fp32=4, fp32r=1 only
when moving>=256, bf16=1 always, fp8e4+DoubleRow=0.5):
  - V / O projections: fp32r operands (moving dims 384 -> already 1 cyc/row).
  - Q/K projections: fp8e4m3 operands with MatmulPerfMode.DoubleRow, feeding
    two 128-row k-subtiles per PE pass ([P, 2, *] slices of the [P, KT, *]
    layout).  Dominant error source: ~1.2e-2 absmax-rel end to end (gate is
    2e-2).  QK_FP8=0 falls back to fp32r (error ~2e-3).
  - Attention path (Q^T/K^T tiles, exp output P, V tiles, Y, Wo): bf16.
    This makes every PV matmul (free=65) and nq=128 score tile 1 cyc/row.
  - exp() reads f32 score PSUM, emits bf16; softmax normalization is a packed
    per-head-pair reciprocal + broadcast multiply on DVE.
  - hw_loop timing programs wrap a LOOP_UNROLL (default 2) iteration body in
    For_i to amortize the ~32us loop-boundary sync.  LOOP_UNROLL=4 measured
    +8us/iter on HW (bigger body hurts more than the halved barrier helps;
    likely instruction-fetch locality), so 2 stays the default.
  - V / O projections emit k-outer / ch-inner so consecutive matmuls
    alternate between the two ch PSUM banks (same stationary back to back):
    measured -6.6us/iter on HW vs the ch-outer ordering.
  - SC_PERHEAD=1 (per-head score tiles, 2 PSUM banks) looks -2us in
    TimelineSim but measured +20us/iter on HW: keep OFF.
  - XV_BF16=1 (default ON): x and Wv in bf16 halve the per-iteration xt
    DMA (6.3 -> 3.1 MB).  Measured -7.1us/iter on HW: the 8 cores share
    HBM bandwidth, so DMA volume matters more than single-core sim says.
  - LOOP_STAGGER=1 + XPIPE=1 (default ON): staggered For_i semaphore
    reset instead of the all-engine barrier, and a cross-trip x prefetch
    (first batch's x tiles peeled before the loop; the body re-DMAs the
    same ring slot mid-trip for the next trip).  Both verified correct on
    the timed hw_loop program; ~-1us/iter each, within run noise.
  - OPIPE=1 (default ON): the last batch's output projection is carried
    across the For_i trip -- yt lives in an explicit [P, 2, KT, S]
    double buffer, the body-start oproj reads slot (len-1)%2 (= the
    previous trip's final yt), and a post-loop flush writes the last
    trip's final batch (also repairing trip 0's garbage pass).  Kills
    the 4-serial-oproj tail at every trip boundary and fills batch-0's
    empty oproj slots.  Measured -3.4us/iter on HW, output-verified
    (relmax 1.205e-2 on the timed program).
  - Y_BF16=1 (bf16 y DMA) measured no gain (y writeback already hidden):
    left OFF to keep its ~3e-3 error headroom.  O_DMA=1 fails an internal
    assert at build.  The remaining gap to sim (~35us) is cross-engine
    chain stalls (sim gap attribution: 19.5us PE idle before V/O proj
    matmuls = ps_mm ring backpressure through the PSUM->SBUF copies;
    15.9us before n=128 matmuls = transposes/scores on their single
    PSUM banks) plus ACT exp occupancy (69us/iter).
  - PSUM_MERGE=1 + B_PV=3 (transpose scratch shares the pv tag ring, 3
    rotating banks replace pv2+yt1) verified correct but measured
    noise-to-slightly-worse on HW: OFF.  PSUM slots pad to full 2KB
    banks per tag-buf, so no sub-bank packing is possible via pools;
    all 8 banks are committed (mm3 + sc2 + pv2 + yt1), which blocks
    B_MM=4 / B_SC=2 -- the PSUM wall is the binding constraint on
    further chain decoupling.
  - fp8 V or O projections are numerically dead: host-sim absmax-rel 4e-2
    vs the 2e-2 gate (vs 1.2e-2 for the current QK-fp8-only config).
Set BASS_MM_F32=1 + QK_FP8=0 for a strict-fp32 fallback.
"""

import contextlib
import os
import sys

import numpy as np

for _p in ("/opt/trn_rl_repo", "/opt/trn_rl_repo/concourse"):
    if _p not in sys.path:
        sys.path.insert(0, _p)

import concourse.bass as bass
import concourse.bacc as bacc
import concourse.mybir as mybir
import concourse.tile as tile

P = 128
E = 768
S = 512
H = 12
D = 64
HP = H // 2          # head pairs
KT = E // P          # 6 feature k-tiles
N_CORES = 8
B_FULL = 32
B_CORE = B_FULL // N_CORES   # 4 batches per core
TOK = B_CORE * S             # 2048 tokens per core
ST = S // P                  # 4 token tiles per sequence
NEG = -1.0e6                 # pre-scale mask bias; exp(0.125 * -1e6) == 0
F32 = mybir.dt.float32

# number of 384-wide chunks for the V / O projections
CH = 2
CHW = E // CH  # 384


def build_program(with_bias: bool, repeat: int = 1, hw_loop: bool = False,
                  r_proj: bool = False, r_scores: bool = False, phases: int = 3,
                  att_bf16: bool = True, qk_fp8: bool | None = None):
    if qk_fp8 is None:
        qk_fp8 = USE_QK_FP8
    PDT = mybir.dt.float32r if r_proj else F32   # proj operands (x, weights)
    BF16 = mybir.dt.bfloat16
    FP8 = mybir.dt.float8e4
    # attention-path operand dtype: qt/kt (scores), pt/md (probs), vs (values),
    # yst/yt (attention out) and wo.  bf16 gets 1 PE cycle/row on ALL matmul
    # shapes (fp32 is 4; fp32r is 4 whenever the moving dim < 256, which hits
    # every PV matmul [free=65] and the nq=128 score tiles).
    ADT = BF16 if att_bf16 else (mybir.dt.float32r if r_scores else F32)
    # fp32r for scores / transpose / oproj: f32(r) matmuls self-load their
    # weights (no separate InstLdweights), cutting ~480 PE instructions per
    # iteration.  Measured +10us on HW (= its exec-cycle cost): the PE is
    # exec-cycle bound, not dispatch bound, so this stays OFF.
    lowinst = os.environ.get("LOW_INST", "0") == "1" and att_bf16
    # fp8 DoubleRow scores: qt/kt stored [P, 2, S] e4m3 with subtile 1
    # pre-zeroed; numerically correct on HW but measured +20us (the doubled
    # moving operand streams at full length), so this stays OFF.
    sc_fp8 = (os.environ.get("SC_FP8", "0") == "1") and qk_fp8 and not lowinst
    # route i=0 score tiles through ps_mm to break the ps_sc serial chain
    sc_split0 = os.environ.get("SC_SPLIT0", "0") == "1" and not sc_fp8
    # per-head [P,S] score tiles, bufs=2 in the same 2 PSUM banks: the two
    # head chains alternate banks, halving the serial scores->exp backbone
    sc_perhead = os.environ.get("SC_PERHEAD", "0") == "1" and not sc_fp8 and not sc_split0
    SCDT = mybir.dt.float32r if lowinst else ADT   # qt/kt (scores operands)
    YSTDT = F32 if lowinst else ADT                # normalize out / transpose in
    YTDT = mybir.dt.float32r if lowinst else ADT   # yt (oproj stationary)
    WODT = (mybir.dt.float32r if lowinst else BF16) if att_bf16 else PDT
    # bf16 x + Wv: halves the per-iteration xt DMA; vproj stays 1 cyc/row.
    xv_bf16 = os.environ.get("XV_BF16", "1") == "1" and att_bf16
    XDT = BF16 if xv_bf16 else PDT
    WVDT = BF16 if xv_bf16 else PDT
    nc = bacc.Bacc(None)
    _eng = {"dve": nc.vector, "act": nc.scalar, "pool": nc.gpsimd, "any": nc.any}
    MASK_ENG = _eng[os.environ.get("MASK_ENG", "dve")].tensor_mul
    MEMSET_ENG = _eng[os.environ.get("MEMSET_ENG", "pool")].memset
    QKCP = _eng[os.environ.get("QKCP_ENG", "any")].tensor_copy
    VCP = _eng[os.environ.get("VCP_ENG", "any")].tensor_copy
    YCP = _eng[os.environ.get("YCP_ENG", "any")].tensor_copy
    OCP = _eng[os.environ.get("OCP_ENG", "any")].tensor_copy
    PV_DIAG_FIRST = os.environ.get("PV_DIAG_FIRST", "0") == "1"

    xt_d = nc.dram_tensor("xt", [E, TOK], XDT, kind="ExternalInput")
    _wnames = ("wv", "wo") if qk_fp8 else ("wq", "wk", "wv", "wo")
    _wdt = {"wq": PDT, "wk": PDT, "wv": WVDT, "wo": WODT}
    w_d = {
        n: nc.dram_tensor(n, [E, E], _wdt[n], kind="ExternalInput")
        for n in _wnames
    }
    consts_d = nc.dram_tensor("consts", [P, 3 * P], F32, kind="ExternalInput")
    if att_bf16:
        # bf16 identity (PE transpose moving operand) + bf16 causal 0/1 mask
        cb_d = nc.dram_tensor("cb", [P, 2 * P], BF16, kind="ExternalInput")
    if qk_fp8:
        xt8_d = nc.dram_tensor("xt8", [E, TOK], FP8, kind="ExternalInput")
        w8_d = {
            n: nc.dram_tensor(n + "8", [E, E], FP8, kind="ExternalInput")
            for n in ("wq", "wk")
        }
    if with_bias:
        bqk_d = nc.dram_tensor("bqk", [P, 2 * KT], F32, kind="ExternalInput")
        bv_d = nc.dram_tensor("bvb", [P, H * (D + 1)], F32, kind="ExternalInput")
        bo_d = nc.dram_tensor("bob", [P, E], F32, kind="ExternalInput")
    # bf16 y output: halves the y DMA (6.3 -> 3.1 MB per iteration); host
    # converts back to f32.  Adds <= ~0.2% per-element rounding on the output.
    y_bf16 = os.environ.get("Y_BF16", "0") == "1" and att_bf16 and not with_bias
    y_d = nc.dram_tensor("y", [TOK, E], BF16 if y_bf16 else F32,
                         kind="ExternalOutput")

    with tile.TileContext(nc) as tc:
        with (
            tc.tile_pool(name="wpool", bufs=1) as wpool,
            tc.tile_pool(name="xpool", bufs=2) as xpool,
            tc.tile_pool(name="qkpool", bufs=int(os.environ.get("B_QK", "3"))) as qkpool,
            tc.tile_pool(name="vpool", bufs=int(os.environ.get("B_VS", "2"))) as vpool,
            tc.tile_pool(name="ppool", bufs=int(os.environ.get("B_PT", "8"))) as ppool,
            tc.tile_pool(name="mdpool", bufs=int(os.environ.get("B_MD", "8"))) as mdpool,
            tc.tile_pool(name="ypool", bufs=4) as ypool,
            tc.tile_pool(name="ytpool", bufs=int(os.environ.get(
                "B_YTP", "4" if os.environ.get("OPIPE", "1") == "1" else "2"))) as ytpool,
            tc.tile_pool(name="opool", bufs=2) as opool,
            tc.tile_pool(name="rpool", bufs=4) as rpool,
            tc.tile_pool(name="ps_mm", bufs=int(os.environ.get("B_MM", "3")), space="PSUM") as ps_mm,
            (tc.tile_pool(name="ps_qk", bufs=1, space="PSUM")
             if os.environ.get("QK_POOL", "0") == "1"
             else contextlib.nullcontext()) as ps_qk,
            tc.tile_pool(name="ps_sc", bufs=int(os.environ.get("B_SC", "2" if (os.environ.get("SC_PERHEAD", "0") == "1") else "1")), space="PSUM") as ps_sc,
            tc.tile_pool(name="ps_pv", bufs=int(os.environ.get("B_PV", "2")), space="PSUM") as ps_pv,
            tc.tile_pool(name="ps_yt", bufs=int(os.environ.get("B_YT", "1")), space="PSUM") as ps_yt,
        ):
            # ---- persistent constants ----
            w_sb = {}
            for n in _wnames:
                t = wpool.tile([P, KT, E], _wdt[n], tag=n)
                nc.sync.dma_start(t[:], w_d[n][:].rearrange("(ko ki) m -> ki ko m", ki=P))
                w_sb[n] = t
            cons = wpool.tile([P, 3 * P], F32, tag="consts")  # masks stay f32
            nc.sync.dma_start(cons[:], consts_d[:])
            ident = cons[:, 0:P]
            mask01 = cons[:, 2 * P : 3 * P]
            if att_bf16:
                cb = wpool.tile([P, 2 * P], mybir.dt.bfloat16, tag="cb")
                nc.sync.dma_start(cb[:], cb_d[:])
                if not lowinst:
                    ident = cb[:, 0:P]
                mask01 = cb[:, P : 2 * P]
            w8_sb = {}
            if qk_fp8:
                for n in ("wq", "wk"):
                    t = wpool.tile([P, KT, E], FP8, tag=n + "8")
                    nc.sync.dma_start(
                        t[:], w8_d[n][:].rearrange("(ko ki) m -> ki ko m", ki=P)
                    )
                    w8_sb[n] = t
            if with_bias:
                bqk = wpool.tile([P, 2 * KT], F32, tag="bqk")
                nc.sync.dma_start(bqk[:], bqk_d[:])
                bvb = wpool.tile([P, H * (D + 1)], F32, tag="bvb")
                nc.sync.dma_start(bvb[:], bv_d[:])
                bob = wpool.tile([P, E], F32, tag="bob")
                nc.sync.dma_start(bob[:], bo_d[:])

            if sc_fp8:
                # pre-zero subtile 1 of every qk pool buffer once; the live
                # copies only ever write subtile 0, so these zeros persist
                for _ in range(int(os.environ.get("B_QK", "3"))):
                    for tag in ("qt", "kt"):
                        tz = qkpool.tile([P, 2, S], FP8, tag=tag, name="tz")
                        MEMSET_ENG(tz[:, 1, :], 0.0)

            xt_r = xt_d[:].rearrange("(ko ki) t -> ki ko t", ki=P)
            if qk_fp8:
                xt8_r = xt8_d[:].rearrange("(ko ki) t -> ki ko t", ki=P)

            xts_t = {}

            def load(pos, b):
                tok0 = (b % B_CORE) * S
                xts = xpool.tile([P, KT, S], XDT, tag="xts")
                nc.sync.dma_start(xts[:], xt_r[:, :, tok0 : tok0 + S])
                xts8 = None
                if qk_fp8:
                    xts8 = xpool.tile([P, KT, S], FP8, tag="xts8")
                    nc.sync.dma_start(xts8[:], xt8_r[:, :, tok0 : tok0 + S])
                xts_t[pos] = (xts, xts8)

            def vproj(b, xts):
                # ---- V projection (token-major, augmented with ones cols) ----
                # k outer / ch inner: the two ch matmuls share the same
                # stationary (xts k-slice), so legalization skips every other
                # InstLdweights (48 -> 24 weight loads per batch).
                vs = []
                for tt in range(ST):
                    v_t = vpool.tile([P, H, D + 1], ADT, tag=f"vs{tt}")
                    MEMSET_ENG(v_t[:, :, D : D + 1], 1.0)
                    pss = [ps_mm.tile([P, S], F32, tag="mm", name=f"psv{c}")
                           for c in range(CH)]
                    for k in range(KT):
                        for ch in range(CH):
                            nc.tensor.matmul(
                                pss[ch][:, :CHW],
                                xts[:, k, tt * P : (tt + 1) * P],
                                w_sb["wv"][:, k, ch * CHW : (ch + 1) * CHW],
                                start=(k == 0),
                                stop=(k == KT - 1),
                            )
                    for ch in range(CH):
                        psc = pss[ch][:, :CHW]
                        hpc = CHW // D  # heads per chunk (6)
                        dst = v_t[:, ch * hpc : (ch + 1) * hpc, 0:D]
                        VCP(out=dst, in_=psc.rearrange("p (h d) -> p h d", d=D))
                    if with_bias:
                        nc.vector.tensor_add(
                            out=v_t[:],
                            in0=v_t[:],
                            in1=bvb[:].rearrange("p (h d) -> p h d", d=D + 1),
                        )
                    vs.append(v_t)
                return vs

            def qk_scores(b, xts, xts8, hp):
                # Q^T / K^T for this head pair (feature tile hp)
                qk = {}
                for name, tag in (("wq", "qt"), ("wk", "kt")):
                    if sc_fp8:
                        dst = qkpool.tile([P, 2, S], FP8, tag=tag)
                    else:
                        dst = qkpool.tile([P, S], SCDT, tag=tag)
                    # QK_POOL=1: qkproj gets its own PSUM bank so its
                    # allocation never backpressures on vproj/oproj copy
                    # evacuations (pair with B_MM=2 to stay in 8 banks)
                    ps = (ps_qk if ps_qk is not None else ps_mm).tile(
                        [P, S], F32, tag="qk" if ps_qk is not None else "mm")
                    if qk_fp8:
                        # fp8 DoubleRow: two 128-row k-subtiles per pass
                        for k in range(0, KT, 2):
                            nc.tensor.matmul(
                                ps[:],
                                w8_sb[name][:, k : k + 2, hp * P : (hp + 1) * P],
                                xts8[:, k : k + 2, :],
                                start=(k == 0),
                                stop=(k == KT - 2),
                                perf_mode=mybir.MatmulPerfMode.DoubleRow,
                            )
                    else:
                        for k in range(KT):
                            nc.tensor.matmul(
                                ps[:],
                                w_sb[name][:, k, hp * P : (hp + 1) * P],
                                xts[:, k, :],
                                start=(k == 0),
                                stop=(k == KT - 1),
                            )
                    if with_bias:
                        col = (0 if name == "wq" else KT) + hp
                        nc.vector.tensor_scalar_add(
                            dst[:, 0, :] if sc_fp8 else dst[:], ps[:],
                            bqk[:, col : col + 1],
                        )
                    else:
                        QKCP(out=dst[:, 0, :] if sc_fp8 else dst[:], in_=ps[:])
                    qk[tag] = dst
                qt, kt = qk["qt"], qk["kt"]

                # scores^T + exp, causal-trimmed per k-tile.  With
                # SC_SPLIT0 the i=0 (nq=512) tiles go through ps_mm per head,
                # so the serial scores->exp chain through the single ps_sc
                # bank loses its heaviest link and the two pools alternate.
                pts = []  # pts[i] = exp(scores^T) [P, 2, Nq] (heads of pair)
                for i in range(ST):
                    nq = S - i * P
                    qoff = i * P
                    if sc_split0 and i == 0:
                        pt = ppool.tile([P, 2, S], ADT, tag="pt")
                        for hh in range(2):
                            ro = hh * D
                            psh = ps_mm.tile([P, S], F32, tag="mm")
                            nc.tensor.matmul(
                                psh[:, 0:nq],
                                kt[ro : ro + D, i * P : (i + 1) * P],
                                qt[ro : ro + D, qoff:S],
                                start=True,
                                stop=True,
                                tile_position=(ro, 0),
                            )
                            nc.scalar.activation(
                                pt[:, hh, 0:nq],
                                psh[:, 0:nq],
                                mybir.ActivationFunctionType.Exp,
                                scale=0.125,
                            )
                    elif sc_perhead:
                        pt = ppool.tile([P, 2, S], ADT, tag="pt")
                        for hh in range(2):
                            ro = hh * D
                            psh = ps_sc.tile([P, S], F32, tag="sc")
                            nc.tensor.matmul(
                                psh[:, 0:nq],
                                kt[ro : ro + D, i * P : (i + 1) * P],
                                qt[ro : ro + D, qoff:S],
                                start=True,
                                stop=True,
                                tile_position=(ro, 0),
                            )
                            nc.scalar.activation(
                                pt[:, hh, 0:nq],
                                psh[:, 0:nq],
                                mybir.ActivationFunctionType.Exp,
                                scale=0.125,
                            )
                    else:
                        ps = ps_sc.tile(
                            [P, 2, 3 * P] if sc_split0 else [P, 2, S],
                            F32, tag="sc",
                        )
                        for hh in range(2):
                            ro = hh * D
                            if sc_fp8:
                                nc.tensor.matmul(
                                    ps[:, hh, 0:nq],
                                    kt[ro : ro + D, :, i * P : (i + 1) * P],
                                    qt[ro : ro + D, :, qoff:S],
                                    start=True,
                                    stop=True,
                                    tile_position=(ro, 0),
                                    perf_mode=mybir.MatmulPerfMode.DoubleRow,
                                )
                            else:
                                nc.tensor.matmul(
                                    ps[:, hh, 0:nq],
                                    kt[ro : ro + D, i * P : (i + 1) * P],
                                    qt[ro : ro + D, qoff:S],
                                    start=True,
                                    stop=True,
                                    tile_position=(ro, 0),
                                )
                        pt = ppool.tile([P, 2, S], ADT, tag="pt")
                        nc.scalar.activation(
                            pt[:, :, 0:nq],
                            ps[:, :, 0:nq],
                            mybir.ActivationFunctionType.Exp,
                            scale=0.125,
                        )
                    # causal mask: zero the upper triangle of the diagonal
                    # block, off the PE->ACT critical path (Pool engine,
                    # post-exp; all-SBUF operands so GpSimd can run it)
                    md = mdpool.tile([P, 2, P], ADT, tag="md")
                    MASK_ENG(
                        out=md[:], in0=pt[:, :, 0:P],
                        in1=mask01[:, None, :].to_broadcast((P, 2, P)),
                    )
                    pts.append((pt, md))
                return pts

            TPOST = os.environ.get("TPOST", "0") == "1"

            def pv_j(hp, pts, vs, j):
                yst = ypool.tile([P, 2, D], YSTDT, tag="yst")
                pv = ps_pv.tile([P, 2, D + 1], F32, tag="pv")
                for hh in range(2):
                    h = 2 * hp + hh
                    order = ([j] + list(range(j))) if PV_DIAG_FIRST else range(j + 1)
                    for ii, i in enumerate(order):
                        pt, md = pts[i]
                        lhsT = (
                            md[:, hh, :]
                            if i == j
                            else pt[:, hh, (j - i) * P : (j - i + 1) * P]
                        )
                        nc.tensor.matmul(
                            pv[:, hh, :],
                            lhsT,
                            vs[i][:, h, :],
                            start=(ii == 0),
                            stop=(ii == j),
                        )
                # one packed reciprocal + one broadcast multiply per
                # (head-pair, q-tile) on DVE, replacing 4 ACT/DVE ops
                r = rpool.tile([P, 2], F32, tag="r")
                nc.vector.reciprocal(r[:], pv[:, :, D])
                nc.vector.tensor_mul(
                    out=yst[:],
                    in0=pv[:, :, 0:D],
                    in1=r[:, :, None].to_broadcast((P, 2, D)),
                )
                return yst

            # PSUM_MERGE=1: allocate the transpose scratch from the pv pool's
            # OWN tag ring (PSUM slots pad to 2KB banks, so pv/ytp share the
            # same slot size).  With B_PV=3 the pv chain and the transposes
            # rotate through 3 banks instead of pv alternating 2 and every
            # transpose serializing on ps_yt's single bank -- PSUM-neutral
            # (3 banks replace the old 2+1).
            PSUM_MERGE = os.environ.get("PSUM_MERGE", "0") == "1"

            def yst_out(hp, yt, j, yst):
                if PSUM_MERGE:
                    yt_ps = ps_pv.tile([P, P], YSTDT, tag="pv", name="yt_ps")
                else:
                    yt_ps = ps_yt.tile([P, P], YSTDT, tag="ytp")
                nc.tensor.transpose(yt_ps[:], yst[:], ident)
                YCP(out=yt[:, hp, j * P : (j + 1) * P], in_=yt_ps[:])

            def pv_block(hp, pts, vs, yt):
                # P @ V_aug accumulated over k-tiles, then normalize,
                # then transpose Y back to feature-major.  With TPOST the
                # transposes of a j-pair are deferred until after both PV
                # chains so they do not head-of-line-block the PE queue
                # while the DVE normalize completes.
                if TPOST:
                    for jp in range(0, ST, 2):
                        ysts = [(j, pv_j(hp, pts, vs, j)) for j in (jp, jp + 1)]
                        for j, yst in ysts:
                            yst_out(hp, yt, j, yst)
                else:
                    for j in range(ST):
                        yst = pv_j(hp, pts, vs, j)
                        yst_out(hp, yt, j, yst)

            # O_DMA needs f32 y (DMA cannot convert PSUM f32 -> bf16)
            O_DMA = (os.environ.get("O_DMA", "0") == "1" and not with_bias
                     and not y_bf16)

            def oproj_tt(b, yt, tt):
                tok0 = (b % B_CORE) * S
                # k outer / ch inner: both ch matmuls share the stationary
                # (yt k-slice) so half the InstLdweights are elided.
                pss = [ps_mm.tile([P, S], F32, tag="mm", name=f"pso{c}")
                       for c in range(CH)]
                for k in range(KT):
                    for ch in range(CH):
                        nc.tensor.matmul(
                            pss[ch][:, :CHW],
                            yt[:, k, tt * P : (tt + 1) * P],
                            w_sb["wo"][:, k, ch * CHW : (ch + 1) * CHW],
                            start=(k == 0),
                            stop=(k == KT - 1),
                        )
                if O_DMA:
                    # DMA y straight out of PSUM, skipping the SBUF bounce
                    for ch in range(CH):
                        nc.sync.dma_start(
                            y_d[
                                tok0 + tt * P : tok0 + (tt + 1) * P,
                                ch * CHW : (ch + 1) * CHW,
                            ],
                            pss[ch][:, :CHW],
                        )
                else:
                    o_sb = opool.tile([P, E], BF16 if y_bf16 else F32, tag="osb")
                    for ch in range(CH):
                        OCP(out=o_sb[:, ch * CHW : (ch + 1) * CHW], in_=pss[ch][:, :CHW])
                    if with_bias:
                        nc.vector.tensor_add(out=o_sb[:], in0=o_sb[:], in1=bob[:])
                    nc.sync.dma_start(
                        y_d[tok0 + tt * P : tok0 + (tt + 1) * P, :], o_sb[:]
                    )

            def run_batches(batches, cross_trip=False, opipe=False):
                # Software-pipelined emission: scores of head-pair hp+1 are
                # emitted before the PV block of hp, so the tensor engine's
                # in-order stream always has matmuls to run while the
                # mask(DVE) -> exp(ACT) -> normalize(DVE) chains drain.
                # cross_trip: position 0's x tiles were loaded by the caller
                # (peel before For_i); the body re-loads them mid-body for
                # the NEXT trip so the post-barrier start never waits on DMA.
                # opipe: the last batch's oproj is carried into the NEXT
                # trip's batch-0 hp slots (yt tiles pre-allocated so the
                # body start can reference the last slot's address); the
                # caller must flush the returned pending_o after the loop,
                # which also repairs trip 0's garbage pass.
                if not cross_trip:
                    load(0, batches[0])
                if opipe:
                    # explicit double buffer (no pool ring): slot idx%2.
                    # Reading slot (len-1)%2 at body start reads the
                    # previous trip's final-batch yt; dependency edges come
                    # from AP-overlap tracking on the shared tensor.
                    ytbuf = wpool.tile([P, 2, KT, S], YTDT, tag="ytbuf")
                    pending_o = (batches[-1],
                                 ytbuf[:, (len(batches) - 1) % 2])
                else:
                    ytbuf = None
                    pending_o = None  # (b, yt) of the previous batch
                for idx, b in enumerate(batches):
                    xts, xts8 = xts_t.pop(idx)
                    vs = vproj(b, xts)
                    yt = (ytbuf[:, idx % 2] if opipe
                          else ytpool.tile([P, KT, S], YTDT, tag="yt"))
                    pts_next = qk_scores(b, xts, xts8, 0)
                    for hp in range(HP):
                        pts_cur = pts_next
                        # previous batch's output projection, one token tile
                        # at a time, spread through the PV chain gaps
                        if pending_o is not None and hp < ST:
                            oproj_tt(*pending_o, hp)
                        if hp == 2:
                            if idx + 1 < len(batches):
                                load(idx + 1, batches[idx + 1])
                            elif cross_trip:
                                # next trip's first batch (same ring slot as
                                # the peel: allocation counts per tag are
                                # equal every trip, so the address matches)
                                load(0, batches[0])
                        if hp + 1 < HP:
                            pts_next = qk_scores(b, xts, xts8, hp + 1)
                        pv_block(hp, pts_cur, vs, yt)
                    pending_o = (b, yt)
                if opipe:
                    return pending_o
                for tt in range(ST):
                    oproj_tt(*pending_o, tt)
                return None

            # hw_loop body covers `unroll` logical iterations to amortize the
            # For_i boundary sync; repeat must be a multiple of unroll.
            unroll = int(os.environ.get("LOOP_UNROLL", "2"))
            # staggered_reset replaces the per-trip all-engine barrier with
            # per-stage semaphore resets staggered through the body;
            # hint_engines adds branch-prefetch hints on the back edge.
            stagger = os.environ.get("LOOP_STAGGER", "1") == "1"
            hints = (list(mybir.ALL_ENGINES)
                     if os.environ.get("LOOP_HINTS", "0") == "1" else ())
            xpipe = os.environ.get("XPIPE", "1") == "1"
            opipe = os.environ.get("OPIPE", "1") == "1"
            if hw_loop and repeat > 1:
                if repeat % unroll != 0:
                    unroll = 1
                body = [b % B_CORE for b in range(B_CORE * unroll)]
                if xpipe:
                    # peel the first x load; the body reloads slot 0 mid-trip
                    # for the next trip so the post-barrier start is DMA-free
                    load(0, body[0])
                with tc.For_i(0, repeat // unroll, 1,
                              staggered_reset=stagger, hint_engines=hints):
                    po = run_batches(body, cross_trip=xpipe, opipe=opipe)
                if stagger:
                    tc.epilogue_barrier()
                if opipe:
                    # post-loop flush: writes the last trip's final-batch y
                    # (and repairs the garbage batch the first trip's
                    # carried-in oproj produced)
                    for tt in range(ST):
                        oproj_tt(*po, tt)
            else:
                run_batches([b % B_CORE for b in range(B_CORE * repeat)])

    nc.compile()
    return nc


def _host_consts():
    ident = np.eye(P, dtype=np.float32)
    k_idx = np.arange(P, dtype=np.int64)[:, None]
    q_idx = np.arange(P, dtype=np.int64)[None, :]
    maskb = np.where(k_idx <= q_idx, 0.0, NEG).astype(np.float32)
    mask01 = (k_idx <= q_idx).astype(np.float32)
    return np.concatenate([ident, maskb, mask01], axis=1)  # [P, 3P]


def _host_consts_bf16():
    import ml_dtypes

    ident = np.eye(P, dtype=np.float32)
    k_idx = np.arange(P, dtype=np.int64)[:, None]
    q_idx = np.arange(P, dtype=np.int64)[None, :]
    mask01 = (k_idx <= q_idx).astype(np.float32)
    return np.concatenate([ident, mask01], axis=1).astype(ml_dtypes.bfloat16)


_PROG_CACHE = {}


# fp32r (relaxed single-pass fp32 matmul, ~2e-4 rel err, 4x PE throughput) is
# used by default; set BASS_MM_F32=1 for strict fp32 matmuls (~2x slower).
USE_F32R = os.environ.get("BASS_MM_F32", "0") != "1"
# fp8e4m3 DoubleRow Q/K projections (2 k-subtiles per PE pass).
USE_QK_FP8 = os.environ.get("QK_FP8", "1") == "1"


def _get_program(with_bias: bool):
    if with_bias not in _PROG_CACHE:
        _PROG_CACHE[with_bias] = build_program(
            with_bias, r_proj=USE_F32R, r_scores=USE_F32R
        )
    return _PROG_CACHE[with_bias]


def make_in_maps(x, Wq, bq, Wk, bk, Wv, bv, Wo, bo, with_bias, att_bf16=True):
    import ml_dtypes

    consts = _host_consts()
    lowinst = os.environ.get("LOW_INST", "0") == "1" and att_bf16
    wo_dt = (np.float32 if lowinst else ml_dtypes.bfloat16) if att_bf16 else np.float32
    xv_bf16 = os.environ.get("XV_BF16", "1") == "1" and att_bf16
    x_dt = ml_dtypes.bfloat16 if xv_bf16 else np.float32
    wv_dt = ml_dtypes.bfloat16 if xv_bf16 else np.float32
    maps = []
    for c in range(N_CORES):
        xc = np.ascontiguousarray(
            x[c * B_CORE : (c + 1) * B_CORE]  # [B_CORE, S, E]
            .reshape(TOK, E)
            .T  # [E, TOK]
        ).astype(np.float32)
        m = {
            "xt": np.ascontiguousarray(xc.astype(x_dt)),
            "wv": np.ascontiguousarray(np.asarray(Wv, np.float32).astype(wv_dt)),
            "wo": np.ascontiguousarray(np.asarray(Wo).astype(wo_dt)),
            "consts": consts,
        }
        if not USE_QK_FP8:
            m["wq"] = np.ascontiguousarray(Wq, dtype=np.float32)
            m["wk"] = np.ascontiguousarray(Wk, dtype=np.float32)
        if att_bf16:
            m["cb"] = _host_consts_bf16()
        if USE_QK_FP8:
            f8 = ml_dtypes.float8_e4m3
            m["xt8"] = np.ascontiguousarray(xc.astype(f8))
            m["wq8"] = np.ascontiguousarray(np.asarray(Wq, np.float32).astype(f8))
            m["wk8"] = np.ascontiguousarray(np.asarray(Wk, np.float32).astype(f8))
        if with_bias:
            bqk = np.concatenate(
                [np.asarray(bq).reshape(KT, P).T, np.asarray(bk).reshape(KT, P).T],
                axis=1,
            ).astype(np.float32)
            bvb = np.zeros((P, H, D + 1), np.float32)
            bvb[:, :, :D] = np.broadcast_to(np.asarray(bv).reshape(H, D), (P, H, D))
            m["bqk"] = np.ascontiguousarray(bqk)
            m["bvb"] = np.ascontiguousarray(bvb.reshape(P, H * (D + 1)))
            m["bob"] = np.ascontiguousarray(
                np.broadcast_to(np.asarray(bo, dtype=np.float32), (P, E))
            )
        maps.append(m)
    return maps


def kernel(x, Wq, bq, Wk, bk, Wv, bv, Wo, bo):
    from concourse.bass_utils import run_bass_kernel_spmd

    x = np.asarray(x, dtype=np.float32)
    with_bias = any(
        float(np.abs(np.asarray(b)).max()) != 0.0 for b in (bq, bk, bv, bo)
    )
    nc = _get_program(with_bias)
    in_maps = make_in_maps(x, Wq, bq, Wk, bk, Wv, bv, Wo, bo, with_bias)
    res = run_bass_kernel_spmd(nc, in_maps, core_ids=list(range(N_CORES)))
    out = np.empty((B_FULL, S, E), dtype=np.float32)
    for c in range(N_CORES):
        yc = np.asarray(res.results[c]["y"], dtype=np.float32)  # may be bf16
        out[c * B_CORE : (c + 1) * B_CORE] = yc.reshape(B_CORE, S, E)
    return out



# revision 34
# speedup vs baseline: 1.1492x; 1.0158x over previous
"""Multi-head causal self-attention (B=32, S=512, E=768, H=12, D=64) on 8 TRN2 cores.

Sharding: pure data-parallel over batch (4 batches per core), no collectives.

Per-core layout strategy:
  - x is fed pre-transposed (feature-major) as xT [E, 2048tok].
  - Q^T, K^T are computed feature-major per head-pair (feature tile == head
    pair):  QT_hp = Wq[:, hp].T @ xT   (lhsT=Wq slice, rhs=xT)
  - V is computed token-major with an extra all-ones column per head
    ("V_aug" [tok, H*(D+1)]); the ones column makes the P@V matmul also
    produce the softmax denominators.
  - scores^T[k,q] = K Q^T computed per (head, k-tile of 128 tokens) with the
    causal-trimmed q range [128*i, 512), both heads of a pair packed into the
    128x128 PE array via tile_position row groups.
  - exp() on ScalarE reads score PSUM directly (1/sqrt(D) folded into exp's
    scale), both heads in one call; the causal mask is a post-exp 0/1
    multiply of just the diagonal 128x128 block on VectorE, kept OFF the
    PE->ACT critical path.
  - P@V: out[q, D+1] accumulated over k-tiles i<=j in PSUM; reciprocal of
    column D (the ones-column sum = softmax denominator) normalizes via a
    ScalarE copy with per-partition scale.
  - Y (token-major) is transposed 128x128 via TensorE back to feature-major
    for the output projection, which lands token-major for a contiguous DMA.
  - Emission is software-pipelined (scores of head-pair hp+1 before the PV
    block of hp; next batch's xT DMA prefetched mid-batch) so the in-order
    engine streams always have independent matmuls to hide the cross-engine
    softmax chains.

Dtype strategy (PE cost = moving-dim size x cycles/row; fp32=4, fp32r=1 only
when moving>=256, bf16=1 always, fp8e4+DoubleRow=0.5):
  - V / O projections: fp32r operands (moving dims 384 -> already 1 cyc/row).
  - Q/K projections: fp8e4m3 operands with MatmulPerfMode.DoubleRow, feeding
    two 128-row k-subtiles per PE pass ([P, 2, *] slices of the [P, KT, *]
    layout).  Dominant error source: ~1.2e-2 absmax-rel end to end (gate is
    2e-2).  QK_FP8=0 falls back to fp32r (error ~2e-3).
  - Attention path (Q^T/K^T tiles, exp output P, V tiles, Y, Wo): bf16.
    This makes every PV matmul (free=65) and nq=128 score tile 1 cyc/row.
  - exp() reads f32 score PSUM, emits bf16; softmax normalization is a packed
    per-head-pair reciprocal + broadcast multiply on DVE.
  - hw_loop timing programs wrap a LOOP_UNROLL (default 2) iteration body in
    For_i to amortize the ~32us loop-boundary sync.  LOOP_UNROLL=4 measured
    +8us/iter on HW (bigger body hurts more than the halved barrier helps;
    likely instruction-fetch locality), so 2 stays the default.
  - V / O projections emit k-outer / ch-inner so consecutive matmuls
    alternate between the two ch PSUM banks (same stationary back to back):
    measured -6.6us/iter on HW vs the ch-outer ordering.
  - SC_PERHEAD=1 (per-head score tiles, 2 PSUM banks) looks -2us in
    TimelineSim but measured +20us/iter on HW: keep OFF.
  - XV_BF16=1 (default ON): x and Wv in bf16 halve the per-iteration xt
    DMA (6.3 -> 3.1 MB).  Measured -7.1us/iter on HW: the 8 cores share
    HBM bandwidth, so DMA volume matters more than single-core sim says.
  - LOOP_STAGGER=1 + XPIPE=1 (default ON): staggered For_i semaphore
    reset instead of the all-engine barrier, and a cross-trip x prefetch
    (first batch's x tiles peeled before the loop; the body re-DMAs the
    same ring slot mid-trip for the next trip).  Both verified correct on
    the timed hw_loop program; ~-1us/iter each, within run noise.
  - OPIPE=1 (default ON): the last batch's output projection is carried
    across the For_i trip -- yt lives in an explicit [P, 2, KT, S]
    double buffer, the body-start oproj reads slot (len-1)%2 (= the
    previous trip's final yt), and a post-loop flush writes the last
    trip's final batch (also repairing trip 0's garbage pass).  Kills
    the 4-serial-oproj tail at every trip boundary and fills batch-0's
    empty oproj slots.  Measured -3.4us/iter on HW, output-verified
    (relmax 1.205e-2 on the timed program).
  - Y_BF16=1 (bf16 y DMA) measured no gain (y writeback already hidden):
    left OFF to keep its ~3e-3 error headroom.  O_DMA=1 fails an internal
    assert at build.  The remaining gap to sim (~35us) is cross-engine
    chain stalls (sim gap attribution: 19.5us PE idle before V/O proj
    matmuls = ps_mm ring backpressure through the PSUM->SBUF copies;
    15.9us before n=128 matmuls = transposes/scores on their single
    PSUM banks) plus ACT exp occupancy (69us/iter).
  - PSUM_MERGE=1 + B_PV=3 (transpose scratch shares the pv tag ring, 3
    rotating banks replace pv2+yt1) verified correct but measured
    noise-to-slightly-worse on HW: OFF.  PSUM slots pad to full 2KB
    banks per tag-buf, so no sub-bank packing is possible via pools;
    all 8 banks are committed (mm3 + sc2 + pv2 + yt1), which blocks
    B_MM=4 / B_SC=2 -- the PSUM wall is the binding constraint on
    further chain decoupling.
  - fp8 V or O projections are numerically dead: host-sim absmax-rel 4e-2
    vs the 2e-2 gate (vs 1.2e-2 for the current QK-fp8-only config).
Set BASS_MM_F32=1 + QK_FP8=0 for a strict-fp32 fallback.
"""

import contextlib
import os
import sys

import numpy as np

for _p in ("/opt/trn_rl_repo", "/opt/trn_rl_repo/concourse"):
    if _p not in sys.path:
        sys.path.insert(0, _p)

import concourse.bass as bass
import concourse.bacc as bacc
import concourse.mybir as mybir
import concourse.tile as tile

P = 128
E = 768
S = 512
H = 12
D = 64
HP = H // 2          # head pairs
KT = E // P          # 6 feature k-tiles
N_CORES = 8
B_FULL = 32
B_CORE = B_FULL // N_CORES   # 4 batches per core
TOK = B_CORE * S             # 2048 tokens per core
ST = S // P                  # 4 token tiles per sequence
NEG = -1.0e6                 # pre-scale mask bias; exp(0.125 * -1e6) == 0
F32 = mybir.dt.float32

# number of 384-wide chunks for the V / O projections
CH = 2
CHW = E // CH  # 384


def build_program(with_bias: bool, repeat: int = 1, hw_loop: bool = False,
                  r_proj: bool = False, r_scores: bool = False, phases: int = 3,
                  att_bf16: bool = True, qk_fp8: bool | None = None):
    if qk_fp8 is None:
        qk_fp8 = USE_QK_FP8
    PDT = mybir.dt.float32r if r_proj else F32   # proj operands (x, weights)
    BF16 = mybir.dt.bfloat16
    FP8 = mybir.dt.float8e4
    # attention-path operand dtype: qt/kt (scores), pt/md (probs), vs (values),
    # yst/yt (attention out) and wo.  bf16 gets 1 PE cycle/row on ALL matmul
    # shapes (fp32 is 4; fp32r is 4 whenever the moving dim < 256, which hits
    # every PV matmul [free=65] and the nq=128 score tiles).
    ADT = BF16 if att_bf16 else (mybir.dt.float32r if r_scores else F32)
    # fp32r for scores / transpose / oproj: f32(r) matmuls self-load their
    # weights (no separate InstLdweights), cutting ~480 PE instructions per
    # iteration.  Measured +10us on HW (= its exec-cycle cost): the PE is
    # exec-cycle bound, not dispatch bound, so this stays OFF.
    lowinst = os.environ.get("LOW_INST", "0") == "1" and att_bf16
    # fp8 DoubleRow scores: qt/kt stored [P, 2, S] e4m3 with subtile 1
    # pre-zeroed; numerically correct on HW but measured +20us (the doubled
    # moving operand streams at full length), so this stays OFF.
    sc_fp8 = (os.environ.get("SC_FP8", "0") == "1") and qk_fp8 and not lowinst
    # route i=0 score tiles through ps_mm to break the ps_sc serial chain
    sc_split0 = os.environ.get("SC_SPLIT0", "0") == "1" and not sc_fp8
    # per-head [P,S] score tiles, bufs=2 in the same 2 PSUM banks: the two
    # head chains alternate banks, halving the serial scores->exp backbone
    sc_perhead = os.environ.get("SC_PERHEAD", "0") == "1" and not sc_fp8 and not sc_split0
    SCDT = mybir.dt.float32r if lowinst else ADT   # qt/kt (scores operands)
    YSTDT = F32 if lowinst else ADT                # normalize out / transpose in
    YTDT = mybir.dt.float32r if lowinst else ADT   # yt (oproj stationary)
    WODT = (mybir.dt.float32r if lowinst else BF16) if att_bf16 else PDT
    # bf16 x + Wv: halves the per-iteration xt DMA; vproj stays 1 cyc/row.
    xv_bf16 = os.environ.get("XV_BF16", "1") == "1" and att_bf16
    XDT = BF16 if xv_bf16 else PDT
    WVDT = BF16 if xv_bf16 else PDT
    nc = bacc.Bacc(None)
    _eng = {"dve": nc.vector, "act": nc.scalar, "pool": nc.gpsimd, "any": nc.any}
    MASK_ENG = _eng[os.environ.get("MASK_ENG", "dve")].tensor_mul
    MEMSET_ENG = _eng[os.environ.get("MEMSET_ENG", "pool")].memset
    QKCP = _eng[os.environ.get("QKCP_ENG", "any")].tensor_copy
    VCP = _eng[os.environ.get("VCP_ENG", "any")].tensor_copy
    YCP = _eng[os.environ.get("YCP_ENG", "any")].tensor_copy
    OCP = _eng[os.environ.get("OCP_ENG", "any")].tensor_copy
    PV_DIAG_FIRST = os.environ.get("PV_DIAG_FIRST", "0") == "1"

    xt_d = nc.dram_tensor("xt", [E, TOK], XDT, kind="ExternalInput")
    _wnames = ("wv", "wo") if qk_fp8 else ("wq", "wk", "wv", "wo")
    _wdt = {"wq": PDT, "wk": PDT, "wv": WVDT, "wo": WODT}
    w_d = {
        n: nc.dram_tensor(n, [E, E], _wdt[n], kind="ExternalInput")
        for n in _wnames
    }
    consts_d = nc.dram_tensor("consts", [P, 3 * P], F32, kind="ExternalInput")
    if att_bf16:
        # bf16 identity (PE transpose moving operand) + bf16 causal 0/1 mask
        cb_d = nc.dram_tensor("cb", [P, 2 * P], BF16, kind="ExternalInput")
    if qk_fp8:
        xt8_d = nc.dram_tensor("xt8", [E, TOK], FP8, kind="ExternalInput")
        w8_d = {
            n: nc.dram_tensor(n + "8", [E, E], FP8, kind="ExternalInput")
            for n in ("wq", "wk")
        }
    if with_bias:
        bqk_d = nc.dram_tensor("bqk", [P, 2 * KT], F32, kind="ExternalInput")
        bv_d = nc.dram_tensor("bvb", [P, H * (D + 1)], F32, kind="ExternalInput")
        bo_d = nc.dram_tensor("bob", [P, E], F32, kind="ExternalInput")
    # bf16 y output: halves the y DMA (6.3 -> 3.1 MB per iteration); host
    # converts back to f32.  Adds <= ~0.2% per-element rounding on the output.
    y_bf16 = os.environ.get("Y_BF16", "0") == "1" and att_bf16 and not with_bias
    y_d = nc.dram_tensor("y", [TOK, E], BF16 if y_bf16 else F32,
                         kind="ExternalOutput")

    with tile.TileContext(nc) as tc:
        with (
            tc.tile_pool(name="wpool", bufs=1) as wpool,
            tc.tile_pool(name="xpool", bufs=2) as xpool,
            tc.tile_pool(name="qkpool", bufs=int(os.environ.get("B_QK", "3"))) as qkpool,
            tc.tile_pool(name="vpool", bufs=int(os.environ.get("B_VS", "2"))) as vpool,
            tc.tile_pool(name="ppool", bufs=int(os.environ.get("B_PT", "8"))) as ppool,
            tc.tile_pool(name="mdpool", bufs=int(os.environ.get("B_MD", "8"))) as mdpool,
            tc.tile_pool(name="ypool", bufs=4) as ypool,
            tc.tile_pool(name="ytpool", bufs=int(os.environ.get(
                "B_YTP", "4" if os.environ.get("OPIPE", "1") == "1" else "2"))) as ytpool,
            tc.tile_pool(name="opool", bufs=2) as opool,
            tc.tile_pool(name="rpool", bufs=4) as rpool,
            tc.tile_pool(name="ps_mm", bufs=int(os.environ.get("B_MM", "3")), space="PSUM") as ps_mm,
            (tc.tile_pool(name="ps_qk", bufs=1, space="PSUM")
             if os.environ.get("QK_POOL", "0") == "1"
             else contextlib.nullcontext()) as ps_qk,
            tc.tile_pool(name="ps_sc", bufs=int(os.environ.get("B_SC", "2" if (os.environ.get("SC_PERHEAD", "0") == "1") else "1")), space="PSUM") as ps_sc,
            tc.tile_pool(name="ps_pv", bufs=int(os.environ.get("B_PV", "2")), space="PSUM") as ps_pv,
            tc.tile_pool(name="ps_yt", bufs=int(os.environ.get("B_YT", "1")), space="PSUM") as ps_yt,
        ):
            # ---- persistent constants ----
            w_sb = {}
            for n in _wnames:
                t = wpool.tile([P, KT, E], _wdt[n], tag=n)
                nc.sync.dma_start(t[:], w_d[n][:].rearrange("(ko ki) m -> ki ko m", ki=P))
                w_sb[n] = t
            cons = wpool.tile([P, 3 * P], F32, tag="consts")  # masks stay f32
            nc.sync.dma_start(cons[:], consts_d[:])
            ident = cons[:, 0:P]
            mask01 = cons[:, 2 * P : 3 * P]
            if att_bf16:
                cb = wpool.tile([P, 2 * P], mybir.dt.bfloat16, tag="cb")
                nc.sync.dma_start(cb[:], cb_d[:])
                if not lowinst:
                    ident = cb[:, 0:P]
                mask01 = cb[:, P : 2 * P]
            w8_sb = {}
            if qk_fp8:
                for n in ("wq", "wk"):
                    t = wpool.tile([P, KT, E], FP8, tag=n + "8")
                    nc.sync.dma_start(
                        t[:], w8_d[n][:].rearrange("(ko ki) m -> ki ko m", ki=P)
                    )
                    w8_sb[n] = t
            if with_bias:
                bqk = wpool.tile([P, 2 * KT], F32, tag="bqk")
                nc.sync.dma_start(bqk[:], bqk_d[:])
                bvb = wpool.tile([P, H * (D + 1)], F32, tag="bvb")
                nc.sync.dma_start(bvb[:], bv_d[:])
                bob = wpool.tile([P, E], F32, tag="bob")
                nc.sync.dma_start(bob[:], bo_d[:])

            if sc_fp8:
                # pre-zero subtile 1 of every qk pool buffer once; the live
                # copies only ever write subtile 0, so these zeros persist
                for _ in range(int(os.environ.get("B_QK", "3"))):
                    for tag in ("qt", "kt"):
                        tz = qkpool.tile([P, 2, S], FP8, tag=tag, name="tz")
                        MEMSET_ENG(tz[:, 1, :], 0.0)

            xt_r = xt_d[:].rearrange("(ko ki) t -> ki ko t", ki=P)
            if qk_fp8:
                xt8_r = xt8_d[:].rearrange("(ko ki) t -> ki ko t", ki=P)

            xts_t = {}

            def load(pos, b):
                tok0 = (b % B_CORE) * S
                xts = xpool.tile([P, KT, S], XDT, tag="xts")
                nc.sync.dma_start(xts[:], xt_r[:, :, tok0 : tok0 + S])
                xts8 = None
                if qk_fp8:
                    xts8 = xpool.tile([P, KT, S], FP8, tag="xts8")
                    nc.sync.dma_start(xts8[:], xt8_r[:, :, tok0 : tok0 + S])
                xts_t[pos] = (xts, xts8)

            def vproj(b, xts):
                # ---- V projection (token-major, augmented with ones cols) ----
                # k outer / ch inner: the two ch matmuls share the same
                # stationary (xts k-slice), so legalization skips every other
                # InstLdweights (48 -> 24 weight loads per batch).
                vs = []
                for tt in range(ST):
                    v_t = vpool.tile([P, H, D + 1], ADT, tag=f"vs{tt}")
                    MEMSET_ENG(v_t[:, :, D : D + 1], 1.0)
                    pss = [ps_mm.tile([P, S], F32, tag="mm", name=f"psv{c}")
                           for c in range(CH)]
                    for k in range(KT):
                        for ch in range(CH):
                            nc.tensor.matmul(
                                pss[ch][:, :CHW],
                                xts[:, k, tt * P : (tt + 1) * P],
                                w_sb["wv"][:, k, ch * CHW : (ch + 1) * CHW],
                                start=(k == 0),
                                stop=(k == KT - 1),
                            )
                    for ch in range(CH):
                        psc = pss[ch][:, :CHW]
                        hpc = CHW // D  # heads per chunk (6)
                        dst = v_t[:, ch * hpc : (ch + 1) * hpc, 0:D]
                        VCP(out=dst, in_=psc.rearrange("p (h d) -> p h d", d=D))
                    if with_bias:
                        nc.vector.tensor_add(
                            out=v_t[:],
                            in0=v_t[:],
                            in1=bvb[:].rearrange("p (h d) -> p h d", d=D + 1),
                        )
                    vs.append(v_t)
                return vs

            def qk_scores(b, xts, xts8, hp):
                # Q^T / K^T for this head pair (feature tile hp)
                qk = {}
                for name, tag in (("wq", "qt"), ("wk", "kt")):
                    if sc_fp8:
                        dst = qkpool.tile([P, 2, S], FP8, tag=tag)
                    else:
                        dst = qkpool.tile([P, S], SCDT, tag=tag)
                    # QK_POOL=1: qkproj gets its own PSUM bank so its
                    # allocation never backpressures on vproj/oproj copy
                    # evacuations (pair with B_MM=2 to stay in 8 banks)
                    ps = (ps_qk if ps_qk is not None else ps_mm).tile(
                        [P, S], F32, tag="qk" if ps_qk is not None else "mm")
                    if qk_fp8:
                        # fp8 DoubleRow: two 128-row k-subtiles per pass
                        for k in range(0, KT, 2):
                            nc.tensor.matmul(
                                ps[:],
                                w8_sb[name][:, k : k + 2, hp * P : (hp + 1) * P],
                                xts8[:, k : k + 2, :],
                                start=(k == 0),
                                stop=(k == KT - 2),
                                perf_mode=mybir.MatmulPerfMode.DoubleRow,
                            )
                    else:
                        for k in range(KT):
                            nc.tensor.matmul(
                                ps[:],
                                w_sb[name][:, k, hp * P : (hp + 1) * P],
                                xts[:, k, :],
                                start=(k == 0),
                                stop=(k == KT - 1),
                            )
                    if with_bias:
                        col = (0 if name == "wq" else KT) + hp
                        nc.vector.tensor_scalar_add(
                            dst[:, 0, :] if sc_fp8 else dst[:], ps[:],
                            bqk[:, col : col + 1],
                        )
                    else:
                        QKCP(out=dst[:, 0, :] if sc_fp8 else dst[:], in_=ps[:])
                    qk[tag] = dst
                qt, kt = qk["qt"], qk["kt"]

                # scores^T + exp, causal-trimmed per k-tile.  With
                # SC_SPLIT0 the i=0 (nq=512) tiles go through ps_mm per head,
                # so the serial scores->exp chain through the single ps_sc
                # bank loses its heaviest link and the two pools alternate.
                pts = []  # pts[i] = exp(scores^T) [P, 2, Nq] (heads of pair)
                for i in range(ST):
                    nq = S - i * P
                    qoff = i * P
                    if sc_split0 and i == 0:
                        pt = ppool.tile([P, 2, S], ADT, tag="pt")
                        for hh in range(2):
                            ro = hh * D
                            psh = ps_mm.tile([P, S], F32, tag="mm")
                            nc.tensor.matmul(
                                psh[:, 0:nq],
                                kt[ro : ro + D, i * P : (i + 1) * P],
                                qt[ro : ro + D, qoff:S],
                                start=True,
                                stop=True,
                                tile_position=(ro, 0),
                            )
                            nc.scalar.activation(
                                pt[:, hh, 0:nq],
                                psh[:, 0:nq],
                                mybir.ActivationFunctionType.Exp,
                                scale=0.125,
                            )
                    elif sc_perhead:
                        pt = ppool.tile([P, 2, S], ADT, tag="pt")
                        for hh in range(2):
                            ro = hh * D
                            psh = ps_sc.tile([P, S], F32, tag="sc")
                            nc.tensor.matmul(
                                psh[:, 0:nq],
                                kt[ro : ro + D, i * P : (i + 1) * P],
                                qt[ro : ro + D, qoff:S],
                                start=True,
                                stop=True,
                                tile_position=(ro, 0),
                            )
                            nc.scalar.activation(
                                pt[:, hh, 0:nq],
                                psh[:, 0:nq],
                                mybir.ActivationFunctionType.Exp,
                                scale=0.125,
                            )
                    else:
                        ps = ps_sc.tile(
                            [P, 2, 3 * P] if sc_split0 else [P, 2, S],
                            F32, tag="sc",
                        )
                        for hh in range(2):
                            ro = hh * D
                            if sc_fp8:
                                nc.tensor.matmul(
                                    ps[:, hh, 0:nq],
                                    kt[ro : ro + D, :, i * P : (i + 1) * P],
                                    qt[ro : ro + D, :, qoff:S],
                                    start=True,
                                    stop=True,
                                    tile_position=(ro, 0),
                                    perf_mode=mybir.MatmulPerfMode.DoubleRow,
                                )
                            else:
                                nc.tensor.matmul(
                                    ps[:, hh, 0:nq],
                                    kt[ro : ro + D, i * P : (i + 1) * P],
                                    qt[ro : ro + D, qoff:S],
                                    start=True,
                                    stop=True,
                                    tile_position=(ro, 0),
                                )
                        pt = ppool.tile([P, 2, S], ADT, tag="pt")
                        nc.scalar.activation(
                            pt[:, :, 0:nq],
                            ps[:, :, 0:nq],
                            mybir.ActivationFunctionType.Exp,
                            scale=0.125,
                        )
                    # causal mask: zero the upper triangle of the diagonal
                    # block, off the PE->ACT critical path (Pool engine,
                    # post-exp; all-SBUF operands so GpSimd can run it)
                    md = mdpool.tile([P, 2, P], ADT, tag="md")
                    MASK_ENG(
                        out=md[:], in0=pt[:, :, 0:P],
                        in1=mask01[:, None, :].to_broadcast((P, 2, P)),
                    )
                    pts.append((pt, md))
                return pts

            TPOST = os.environ.get("TPOST", "0") == "1"

            def pv_j(hp, pts, vs, j):
                yst = ypool.tile([P, 2, D], YSTDT, tag="yst")
                pv = ps_pv.tile([P, 2, D + 1], F32, tag="pv")
                for hh in range(2):
                    h = 2 * hp + hh
                    order = ([j] + list(range(j))) if PV_DIAG_FIRST else range(j + 1)
                    for ii, i in enumerate(order):
                        pt, md = pts[i]
                        lhsT = (
                            md[:, hh, :]
                            if i == j
                            else pt[:, hh, (j - i) * P : (j - i + 1) * P]
                        )
                        nc.tensor.matmul(
                            pv[:, hh, :],
                            lhsT,
                            vs[i][:, h, :],
                            start=(ii == 0),
                            stop=(ii == j),
                        )
                # one packed reciprocal + one broadcast multiply per
                # (head-pair, q-tile) on DVE, replacing 4 ACT/DVE ops
                r = rpool.tile([P, 2], F32, tag="r")
                nc.vector.reciprocal(r[:], pv[:, :, D])
                nc.vector.tensor_mul(
                    out=yst[:],
                    in0=pv[:, :, 0:D],
                    in1=r[:, :, None].to_broadcast((P, 2, D)),
                )
                return yst

            # PSUM_MERGE=1: allocate the transpose scratch from the pv pool's
            # OWN tag ring (PSUM slots pad to 2KB banks, so pv/ytp share the
            # same slot size).  With B_PV=3 the pv chain and the transposes
            # rotate through 3 banks instead of pv alternating 2 and every
            # transpose serializing on ps_yt's single bank -- PSUM-neutral
            # (3 banks replace the old 2+1).
            PSUM_MERGE = os.environ.get("PSUM_MERGE", "0") == "1"

            def yst_out(hp, yt, j, yst):
                if PSUM_MERGE:
                    yt_ps = ps_pv.tile([P, P], YSTDT, tag="pv", name="yt_ps")
                else:
                    yt_ps = ps_yt.tile([P, P], YSTDT, tag="ytp")
                nc.tensor.transpose(yt_ps[:], yst[:], ident)
                YCP(out=yt[:, hp, j * P : (j + 1) * P], in_=yt_ps[:])

            def pv_block(hp, pts, vs, yt):
                # P @ V_aug accumulated over k-tiles, then normalize,
                # then transpose Y back to feature-major.  With TPOST the
                # transposes of a j-pair are deferred until after both PV
                # chains so they do not head-of-line-block the PE queue
                # while the DVE normalize completes.
                if TPOST:
                    for jp in range(0, ST, 2):
                        ysts = [(j, pv_j(hp, pts, vs, j)) for j in (jp, jp + 1)]
                        for j, yst in ysts:
                            yst_out(hp, yt, j, yst)
                else:
                    for j in range(ST):
                        yst = pv_j(hp, pts, vs, j)
                        yst_out(hp, yt, j, yst)

            # O_DMA needs f32 y (DMA cannot convert PSUM f32 -> bf16)
            O_DMA = (os.environ.get("O_DMA", "0") == "1" and not with_bias
                     and not y_bf16)

            def oproj_tt(b, yt, tt):
                tok0 = (b % B_CORE) * S
                # k outer / ch inner: both ch matmuls share the stationary
                # (yt k-slice) so half the InstLdweights are elided.
                pss = [ps_mm.tile([P, S], F32, tag="mm", name=f"pso{c}")
                       for c in range(CH)]
                for k in range(KT):
                    for ch in range(CH):
                        nc.tensor.matmul(
                            pss[ch][:, :CHW],
                            yt[:, k, tt * P : (tt + 1) * P],
                            w_sb["wo"][:, k, ch * CHW : (ch + 1) * CHW],
                            start=(k == 0),
                            stop=(k == KT - 1),
                        )
                if O_DMA:
                    # DMA y straight out of PSUM, skipping the SBUF bounce
                    for ch in range(CH):
                        nc.sync.dma_start(
                            y_d[
                                tok0 + tt * P : tok0 + (tt + 1) * P,
                                ch * CHW : (ch + 1) * CHW,
                            ],
                            pss[ch][:, :CHW],
                        )
                else:
                    o_sb = opool.tile([P, E], BF16 if y_bf16 else F32, tag="osb")
                    for ch in range(CH):
                        OCP(out=o_sb[:, ch * CHW : (ch + 1) * CHW], in_=pss[ch][:, :CHW])
                    if with_bias:
                        nc.vector.tensor_add(out=o_sb[:], in0=o_sb[:], in1=bob[:])
                    nc.sync.dma_start(
                        y_d[tok0 + tt * P : tok0 + (tt + 1) * P, :], o_sb[:]
                    )

            def run_batches(batches, cross_trip=False, opipe=False):
                # Software-pipelined emission: scores of head-pair hp+1 are
                # emitted before the PV block of hp, so the tensor engine's
                # in-order stream always has matmuls to run while the
                # mask(DVE) -> exp(ACT) -> normalize(DVE) chains drain.
                # cross_trip: position 0's x tiles were loaded by the caller
                # (peel before For_i); the body re-loads them mid-body for
                # the NEXT trip so the post-barrier start never waits on DMA.
                # opipe: the last batch's oproj is carried into the NEXT
                # trip's batch-0 hp slots (yt tiles pre-allocated so the
                # body start can reference the last slot's address); the
                # caller must flush the returned pending_o after the loop,
                # which also repairs trip 0's garbage pass.
                if not cross_trip:
                    load(0, batches[0])
                if opipe:
                    # explicit double buffer (no pool ring): slot idx%2.
                    # Reading slot (len-1)%2 at body start reads the
                    # previous trip's final-batch yt; dependency edges come
                    # from AP-overlap tracking on the shared tensor.
                    ytbuf = wpool.tile([P, 2, KT, S], YTDT, tag="ytbuf")
                    pending_o = (batches[-1],
                                 ytbuf[:, (len(batches) - 1) % 2])
                else:
                    ytbuf = None
                    pending_o = None  # (b, yt) of the previous batch
                qk_first = os.environ.get("QK_FIRST", "0") == "1"
                for idx, b in enumerate(batches):
                    xts, xts8 = xts_t.pop(idx)
                    # QK_FIRST: emit hp=0's qk+scores BEFORE the 48 vproj
                    # matmuls so exp(0) starts ~7us earlier in the in-order
                    # PE stream, overlapping vproj instead of trailing it
                    if qk_first:
                        pts_next = qk_scores(b, xts, xts8, 0)
                        vs = vproj(b, xts)
                    else:
                        vs = vproj(b, xts)
                        pts_next = qk_scores(b, xts, xts8, 0)
                    yt = (ytbuf[:, idx % 2] if opipe
                          else ytpool.tile([P, KT, S], YTDT, tag="yt"))
                    for hp in range(HP):
                        pts_cur = pts_next
                        # previous batch's output projection, one token tile
                        # at a time, spread through the PV chain gaps
                        if pending_o is not None and hp < ST:
                            oproj_tt(*pending_o, hp)
                        if hp == 2:
                            if idx + 1 < len(batches):
                                load(idx + 1, batches[idx + 1])
                            elif cross_trip:
                                # next trip's first batch (same ring slot as
                                # the peel: allocation counts per tag are
                                # equal every trip, so the address matches)
                                load(0, batches[0])
                        if hp + 1 < HP:
                            pts_next = qk_scores(b, xts, xts8, hp + 1)
                        pv_block(hp, pts_cur, vs, yt)
                    pending_o = (b, yt)
                if opipe:
                    return pending_o
                for tt in range(ST):
                    oproj_tt(*pending_o, tt)
                return None

            # hw_loop body covers `unroll` logical iterations to amortize the
            # For_i boundary sync; repeat must be a multiple of unroll.
            unroll = int(os.environ.get("LOOP_UNROLL", "2"))
            # staggered_reset replaces the per-trip all-engine barrier with
            # per-stage semaphore resets staggered through the body;
            # hint_engines adds branch-prefetch hints on the back edge.
            stagger = os.environ.get("LOOP_STAGGER", "1") == "1"
            hints = (list(mybir.ALL_ENGINES)
                     if os.environ.get("LOOP_HINTS", "0") == "1" else ())
            xpipe = os.environ.get("XPIPE", "1") == "1"
            opipe = os.environ.get("OPIPE", "1") == "1"
            if hw_loop and repeat > 1:
                if repeat % unroll != 0:
                    unroll = 1
                body = [b % B_CORE for b in range(B_CORE * unroll)]
                if xpipe:
                    # peel the first x load; the body reloads slot 0 mid-trip
                    # for the next trip so the post-barrier start is DMA-free
                    load(0, body[0])
                with tc.For_i(0, repeat // unroll, 1,
                              staggered_reset=stagger, hint_engines=hints):
                    po = run_batches(body, cross_trip=xpipe, opipe=opipe)
                if stagger:
                    tc.epilogue_barrier()
                if opipe:
                    # post-loop flush: writes the last trip's final-batch y
                    # (and repairs the garbage batch the first trip's
                    # carried-in oproj produced)
                    for tt in range(ST):
                        oproj_tt(*po, tt)
            else:
                run_batches([b % B_CORE for b in range(B_CORE * repeat)])

    nc.compile()
    return nc


def _host_consts():
    ident = np.eye(P, dtype=np.float32)
    k_idx = np.arange(P, dtype=np.int64)[:, None]
    q_idx = np.arange(P, dtype=np.int64)[None, :]
    maskb = np.where(k_idx <= q_idx, 0.0, NEG).astype(np.float32)
    mask01 = (k_idx <= q_idx).astype(np.float32)
    return np.concatenate([ident, maskb, mask01], axis=1)  # [P, 3P]


def _host_consts_bf16():
    import ml_dtypes

    ident = np.eye(P, dtype=np.float32)
    k_idx = np.arange(P, dtype=np.int64)[:, None]
    q_idx = np.arange(P, dtype=np.int64)[None, :]
    mask01 = (k_idx <= q_idx).astype(np.float32)
    return np.concatenate([ident, mask01], axis=1).astype(ml_dtypes.bfloat16)


_PROG_CACHE = {}


# fp32r (relaxed single-pass fp32 matmul, ~2e-4 rel err, 4x PE throughput) is
# used by default; set BASS_MM_F32=1 for strict fp32 matmuls (~2x slower).
USE_F32R = os.environ.get("BASS_MM_F32", "0") != "1"
# fp8e4m3 DoubleRow Q/K projections (2 k-subtiles per PE pass).
USE_QK_FP8 = os.environ.get("QK_FP8", "1") == "1"


def _get_program(with_bias: bool):
    if with_bias not in _PROG_CACHE:
        _PROG_CACHE[with_bias] = build_program(
            with_bias, r_proj=USE_F32R, r_scores=USE_F32R
        )
    return _PROG_CACHE[with_bias]


def make_in_maps(x, Wq, bq, Wk, bk, Wv, bv, Wo, bo, with_bias, att_bf16=True):
    import ml_dtypes

    consts = _host_consts()
    lowinst = os.environ.get("LOW_INST", "0") == "1" and att_bf16
    wo_dt = (np.float32 if lowinst else ml_dtypes.bfloat16) if att_bf16 else np.float32
    xv_bf16 = os.environ.get("XV_BF16", "1") == "1" and att_bf16
    x_dt = ml_dtypes.bfloat16 if xv_bf16 else np.float32
    wv_dt = ml_dtypes.bfloat16 if xv_bf16 else np.float32
    maps = []
    for c in range(N_CORES):
        xc = np.ascontiguousarray(
            x[c * B_CORE : (c + 1) * B_CORE]  # [B_CORE, S, E]
            .reshape(TOK, E)
            .T  # [E, TOK]
        ).astype(np.float32)
        m = {
            "xt": np.ascontiguousarray(xc.astype(x_dt)),
            "wv": np.ascontiguousarray(np.asarray(Wv, np.float32).astype(wv_dt)),
            "wo": np.ascontiguousarray(np.asarray(Wo).astype(wo_dt)),
            "consts": consts,
        }
        if not USE_QK_FP8:
            m["wq"] = np.ascontiguousarray(Wq, dtype=np.float32)
            m["wk"] = np.ascontiguousarray(Wk, dtype=np.float32)
        if att_bf16:
            m["cb"] = _host_consts_bf16()
        if USE_QK_FP8:
            f8 = ml_dtypes.float8_e4m3
            m["xt8"] = np.ascontiguousarray(xc.astype(f8))
            m["wq8"] = np.ascontiguousarray(np.asarray(Wq, np.float32).astype(f8))
            m["wk8"] = np.ascontiguousarray(np.asarray(Wk, np.float32).astype(f8))
        if with_bias:
            bqk = np.concatenate(
                [np.asarray(bq).reshape(KT, P).T, np.asarray(bk).reshape(KT, P).T],
                axis=1,
            ).astype(np.float32)
            bvb = np.zeros((P, H, D + 1), np.float32)
            bvb[:, :, :D] = np.broadcast_to(np.asarray(bv).reshape(H, D), (P, H, D))
            m["bqk"] = np.ascontiguousarray(bqk)
            m["bvb"] = np.ascontiguousarray(bvb.reshape(P, H * (D + 1)))
            m["bob"] = np.ascontiguousarray(
                np.broadcast_to(np.asarray(bo, dtype=np.float32), (P, E))
            )
        maps.append(m)
    return maps


def kernel(x, Wq, bq, Wk, bk, Wv, bv, Wo, bo):
    from concourse.bass_utils import run_bass_kernel_spmd

    x = np.asarray(x, dtype=np.float32)
    with_bias = any(
        float(np.abs(np.asarray(b)).max()) != 0.0 for b in (bq, bk, bv, bo)
    )
    nc = _get_program(with_bias)
    in_maps = make_in_maps(x, Wq, bq, Wk, bk, Wv, bv, Wo, bo, with_bias)
    res = run_bass_kernel_spmd(nc, in_maps, core_ids=list(range(N_CORES)))
    out = np.empty((B_FULL, S, E), dtype=np.float32)
    for c in range(N_CORES):
        yc = np.asarray(res.results[c]["y"], dtype=np.float32)  # may be bf16
        out[c * B_CORE : (c + 1) * B_CORE] = yc.reshape(B_CORE, S, E)
    return out

